# revision 1
# baseline (speedup 1.0000x reference)
"""Trainium2 Bass kernel for ExternalEmbeddingSelfAttention.

Computation (per batch b):
    q     = hs @ Wq + bq           [S,H]
    k_tok = hs @ Wk + bk           [S,H]
    v_tok = hs @ Wv + bv           [S,H]
    k_ext = ext @ Wk + bk          [E,H]
    v_ext = ext @ Wv + bv          [E,H]
    s_self[t] = q[t] . k_tok[t]                (per-token self score)
    s_ext = q @ k_ext^T            [S,E]
    probs = softmax([s_self, s_ext], axis=-1)  (no 1/sqrt(d) scaling)
    out   = probs[:,0:1]*v_tok + probs[:,1:] @ v_ext

Sharding: 8 cores, each takes 1024 contiguous tokens of the flattened
(B*S, H) token axis (core i -> batch i//2, S-half i%2).  Each core also
computes its batch's external projections (duplicated across the 2 cores
sharing a batch; minor cost).

Device algorithm (per core, T=1024 tokens):
  - Host pre-transposes the hs/ext shards (xT=[H,T], eT=[H,E]) so every
    matmul operand is already in lhsT/rhs layout.
  - QT = Wq^T @ xT (+bq)        hidden-major [H, T]  (f32 for score precision)
  - KxT = Wk^T @ eT (+bk)       hidden-major [H, E]  (f32)
  - s_self accumulated row-oriented [2, T] on PE via ones^T @ (K_tok^T*QT)
    (single PSUM accumulation group; start=True clears a whole PSUM bank,
    so per-column groups in one bank are illegal), then transposed to
    per-token columns.
  - Vx = ext @ Wv (+bv)         E-major [E, H]  (fp32r)
  - Attention per 128-token tile: s_ext = QT.T @ KxT (true fp32 matmul,
    4-pass); softmax with the self column folded in; unnormalized probs
    are PE-transposed; ctx PSUM accumulates BOTH the ext context
    (Pt.T @ Vx) and the self term ((xT * p_self_bcast).T @ Wv + p_self x bv)
    in one group; the final PSUM->SBUF copy applies the 1/Z scale.

All V-side matmuls run as float32r (FP22 multiply, fp32 accumulate) at
full PE rate; the score pipeline (s_ext) runs true fp32.
"""

import numpy as np

B, S, E, H = 4, 2048, 512, 1024
NCORES = 8
T = (B * S) // NCORES  # tokens per core = 1024

_RUNNER_CACHE = {}
_WSTREAM_BUFS = 2

_ONESL = np.ones((1, 128), dtype=np.float32)
_ONESC = np.ones((128, 2), dtype=np.float32)
_EYE = np.eye(128, dtype=np.float32)


# --------------------------------------------------------------------------
# device kernel emission
# --------------------------------------------------------------------------

def _emit(nc, tc, ctx, T, H, E, reps=1):
    import contextlib
    import concourse.mybir as mybir

    f32 = mybir.dt.float32
    f32r = mybir.dt.float32r
    Alu = mybir.AluOpType
    Act = mybir.ActivationFunctionType
    X = mybir.AxisListType.X

    KC = H // 128          # contraction chunks over h_in; also h_out tiles
    NT = T // 128          # token tiles
    NE = E // 128          # ext tiles
    WH = min(512, H)       # h_out free-dim chunk
    NH = H // WH
    WT = min(512, T)       # token free-dim chunk
    NTW = T // WT
    assert E <= 512, "s_ext PSUM group assumes E <= 512"

    xT_d = nc.declare_dram_parameter("xT", [H, T], f32, isOutput=False)
    eT_d = nc.declare_dram_parameter("eT", [H, E], f32, isOutput=False)
    wq_d = nc.declare_dram_parameter("Wq", [H, H], f32, isOutput=False)
    wk_d = nc.declare_dram_parameter("Wk", [H, H], f32, isOutput=False)
    wv_d = nc.declare_dram_parameter("Wv", [H, H], f32, isOutput=False)
    bqc_d = nc.declare_dram_parameter("bqc", [128, KC], f32, isOutput=False)
    bkc_d = nc.declare_dram_parameter("bkc", [128, KC], f32, isOutput=False)
    bvr_d = nc.declare_dram_parameter("bvr", [1, H], f32, isOutput=False)
    onesl_d = nc.declare_dram_parameter("onesl", [1, 128], f32, isOutput=False)
    onesc_d = nc.declare_dram_parameter("onesc", [128, 2], f32, isOutput=False)
    eye_d = nc.declare_dram_parameter("eye", [128, 128], f32, isOutput=False)
    out_d = nc.declare_dram_parameter("out", [T, H], f32, isOutput=True)

    cp = ctx.enter_context(tc.tile_pool(name="cp", bufs=1))
    wstream = ctx.enter_context(tc.tile_pool(name="wstream", bufs=_WSTREAM_BUFS))
    small = ctx.enter_context(tc.tile_pool(name="small", bufs=3))

    # ---- persistent SBUF tensors -----------------------------------------
    QT = cp.tile([128, KC * T], f32r, name="QT")    # Q^T hidden-major
    KxT = cp.tile([128, KC * E], f32r, name="KxT")  # K_ext^T hidden-major
    Vx = cp.tile([128, NE * H], f32r, name="Vx")    # V_ext E-major
    xT = cp.tile([128, KC * T], f32r, name="xT")    # chunk kc: cols [kc*T,+T)
    Wv = cp.tile([128, KC * H], f32r, name="Wv")
    ss_col = cp.tile([128, 2 * NT], f32, name="ss_col")
    ss_row = cp.tile([2, T], f32, name="ss_row")
    bqc = cp.tile([128, KC], f32, name="bqc")
    bkc = cp.tile([128, KC], f32, name="bkc")
    bvr = cp.tile([1, H], f32r, name="bvr")
    ones_l = cp.tile([1, 128], f32r, name="ones_l")
    ones_c = cp.tile([128, 2], f32r, name="ones_c")
    ident = cp.tile([128, 128], f32r, name="ident")

    for kc in range(KC):
        nc.sync.dma_start(xT[:, kc * T:(kc + 1) * T],
                          xT_d[kc * 128:(kc + 1) * 128, :].bitcast(f32r))
    nc.sync.dma_start(bqc[:], bqc_d[:])
    nc.sync.dma_start(bkc[:], bkc_d[:])
    nc.sync.dma_start(bvr[:], bvr_d[:].bitcast(f32r))
    nc.sync.dma_start(ones_l[:], onesl_d[:].bitcast(f32r))
    nc.sync.dma_start(ones_c[:], onesc_d[:].bitcast(f32r))
    nc.sync.dma_start(ident[:], eye_d[:].bitcast(f32r))
    # bv broadcast to all 128 partitions (the value bias is shared by every
    # attended value and probs sum to 1, so it adds once at the end)
    bvb = cp.tile([128, H], f32, name="bvb")
    with tc.tile_pool(name="ps_bv", bufs=2, space="PSUM") as ps_bv:
        for n in range(NH):
            pbv = ps_bv.tile([128, WH], f32, name="pbv")
            nc.tensor.matmul(pbv[:], ones_l[:], bvr[:, n * WH:(n + 1) * WH],
                             start=True, stop=True)
            nc.vector.tensor_copy(bvb[:, n * WH:(n + 1) * WH], pbv[:])
    for kc in range(KC):
        nc.scalar.dma_start(Wv[:, kc * H:(kc + 1) * H],
                            wv_d[kc * 128:(kc + 1) * 128, :].bitcast(f32r))

    loop_cm = tc.For_i(0, reps, 1) if reps > 1 else contextlib.nullcontext()
    with loop_cm:
        # ---- phase Q: QT = Wq^T @ xT (+bq) -------------------------------
        with tc.tile_pool(name="ps_q", bufs=4, space="PSUM") as ps_q:
            wq_r = wq_d.rearrange("(kc p) h -> p kc h", p=128)
            for ho in range(KC):
                psq = [ps_q.tile([128, WT], f32, name="psq")
                       for _ in range(NTW)]
                w = wstream.tile([128, KC * 128], f32r, name="w")
                nc.sync.dma_start(
                    w[:].rearrange("p (kc c) -> p kc c", c=128),
                    wq_r[:, :, ho * 128:(ho + 1) * 128].bitcast(f32r))
                for kc in range(KC):
                    for n in range(NTW):
                        nc.tensor.matmul(
                            psq[n][:], w[:, kc * 128:(kc + 1) * 128],
                            xT[:, kc * T + n * WT: kc * T + (n + 1) * WT],
                            start=(kc == 0), stop=(kc == KC - 1))
                for n in range(NTW):
                    nc.vector.tensor_scalar_add(
                        QT[:, ho * T + n * WT: ho * T + (n + 1) * WT],
                        psq[n][:], bqc[:, ho:ho + 1])

        # ---- phase K + s_self, and V_ext (uses eT) -----------------------
        with tc.tile_pool(name="cp_e", bufs=1) as cp_e:
            eT = cp_e.tile([128, KC * E], f32r, name="eT")
            for kc in range(KC):
                nc.scalar.dma_start(
                    eT[:, kc * E:(kc + 1) * E],
                    eT_d[kc * 128:(kc + 1) * 128, :].bitcast(f32r))

            with tc.tile_pool(name="ps_ke", bufs=2, space="PSUM") as ps_ke, \
                 tc.tile_pool(name="ps_kt", bufs=4, space="PSUM") as ps_kt, \
                 tc.tile_pool(name="ps_ss", bufs=1, space="PSUM") as ps_ss, \
                 tc.tile_pool(name="work_k", bufs=2) as work_k:
                sself = ps_ss.tile([2, T], f32, name="sself")
                wk_r = wk_d.rearrange("(kc p) h -> p kc h", p=128)
                for ho in range(KC):
                    pske = ps_ke.tile([128, E], f32, name="pske")
                    pskt = [ps_kt.tile([128, WT], f32, name="pskt")
                            for _ in range(NTW)]
                    w = wstream.tile([128, KC * 128], f32r, name="w")
                    nc.sync.dma_start(
                        w[:].rearrange("p (kc c) -> p kc c", c=128),
                        wk_r[:, :, ho * 128:(ho + 1) * 128].bitcast(f32r))
                    for kc in range(KC):
                        wc = w[:, kc * 128:(kc + 1) * 128]
                        nc.tensor.matmul(pske[:], wc,
                                         eT[:, kc * E:(kc + 1) * E],
                                         start=(kc == 0), stop=(kc == KC - 1))
                        for n in range(NTW):
                            nc.tensor.matmul(
                                pskt[n][:], wc,
                                xT[:, kc * T + n * WT: kc * T + (n + 1) * WT],
                                start=(kc == 0), stop=(kc == KC - 1))
                    nc.vector.tensor_scalar_add(KxT[:, ho * E:(ho + 1) * E],
                                                pske[:], bkc[:, ho:ho + 1])
                    # d = (k_tok^T + bk) * QT, straight from PSUM
                    d = work_k.tile([128, T], f32r, name="d")
                    for n in range(NTW):
                        nc.vector.scalar_tensor_tensor(
                            d[:, n * WT:(n + 1) * WT], pskt[n][:],
                            bkc[:, ho:ho + 1],
                            QT[:, ho * T + n * WT: ho * T + (n + 1) * WT],
                            Alu.add, Alu.mult)
                    for n in range(NTW):
                        nc.tensor.matmul(sself[:, n * WT:(n + 1) * WT],
                                         ones_c[:], d[:, n * WT:(n + 1) * WT],
                                         start=(ho == 0), stop=(ho == KC - 1))
                nc.vector.tensor_copy(ss_row[:], sself[:])

            # s_self [2, T] -> per-token columns ss_col[:, 2m]
            with tc.tile_pool(name="ps_tr0", bufs=2, space="PSUM") as ps_tr0:
                for m in range(NT):
                    pst2 = ps_tr0.tile([128, 2], f32, name="pst2")
                    nc.tensor.transpose(pst2[:],
                                        ss_row[:, m * 128:(m + 1) * 128],
                                        ident[0:2, 0:2].bitcast(f32))
                    nc.vector.tensor_copy(ss_col[:, 2 * m:2 * m + 2], pst2[:])

            # ---- phase V_ext: Vx = (eT)^T @ Wv (+bv), E-major ------------
            with tc.tile_pool(name="ps_ve", bufs=4, space="PSUM") as ps_ve:
                for eo in range(NE):
                    psv = [ps_ve.tile([128, WH], f32, name="psv")
                           for _ in range(NH)]
                    for kc in range(KC):
                        lhsT = eT[:, kc * E + eo * 128: kc * E + (eo + 1) * 128]
                        for n in range(NH):
                            nc.tensor.matmul(
                                psv[n][:], lhsT,
                                Wv[:, kc * H + n * WH: kc * H + (n + 1) * WH],
                                start=(kc == 0), stop=(kc == KC - 1))
                    for n in range(NH):
                        nc.scalar.copy(
                            Vx[:, eo * H + n * WH: eo * H + (n + 1) * WH],
                            psv[n][:])

        # ---- attention per token tile ------------------------------------
        with tc.tile_pool(name="ps_s", bufs=2, space="PSUM") as ps_att, \
             tc.tile_pool(name="ps_tr", bufs=2, space="PSUM") as ps_tr, \
             tc.tile_pool(name="ps_c", bufs=2, space="PSUM") as ps_c, \
             tc.tile_pool(name="work_a", bufs=2) as work_a, \
             tc.tile_pool(name="work_s", bufs=3) as work_s:
            for m in range(NT):
                # s_ext = Q^T.T @ K_ext^T  -> [128 tokens, E]  (true fp32)
                ps_s = ps_att.tile([128, E], f32, name="ps_s")
                for kc in range(KC):
                    nc.tensor.matmul(
                        ps_s[:],
                        QT[:, kc * T + m * 128: kc * T + (m + 1) * 128],
                        KxT[:, kc * E:(kc + 1) * E],
                        start=(kc == 0), stop=(kc == KC - 1))

                nmx = small.tile([128, 1], f32, name="nmx")
                nc.vector.tensor_reduce(nmx[:], ps_s[:], axis=X, op=Alu.max,
                                        negate=True)
                nself = small.tile([128, 1], f32, name="nself")
                nc.vector.tensor_scalar_mul(nself[:],
                                            ss_col[:, 2 * m:2 * m + 1], -1.0)
                nmx2 = small.tile([128, 1], f32, name="nmx2")
                nc.vector.tensor_tensor(nmx2[:], nmx[:], nself[:], Alu.min)

                # unnormalized probs; Z accumulated on the fly
                pe = work_a.tile([128, E], f32r, name="pe")
                Ze = small.tile([128, 1], f32, name="Ze")
                nc.scalar.activation(pe[:], ps_s[:], Act.Exp, bias=nmx2[:],
                                     scale=1.0, accum_out=Ze[:])
                p_self = small.tile([128, 1], f32, name="p_self")
                nc.scalar.activation(p_self[:], ss_col[:, 2 * m:2 * m + 1],
                                     Act.Exp, bias=nmx2[:], scale=1.0)
                Zt = small.tile([128, 1], f32, name="Zt")
                nc.vector.tensor_tensor(Zt[:], Ze[:], p_self[:], Alu.add)
                r = small.tile([128, 1], f32, name="r")
                nc.vector.reciprocal(r[:], Zt[:])

                # p_self as a row + broadcast to all partitions (for the
                # per-column scaling of the v_tok lhsT)
                psr_ps = ps_tr.tile([1, 128], f32, name="psr_ps", bufs=1)
                nc.tensor.transpose(psr_ps[:], p_self[:].bitcast(f32),
                                    ident[:].bitcast(f32))
                psr = work_s.tile([1, 128], f32r, name="psr")
                nc.vector.tensor_copy(psr[:], psr_ps[:])
                bc_ps = ps_tr.tile([128, 128], f32, name="bc_ps", bufs=1)
                nc.tensor.matmul(bc_ps[:], ones_l[:], psr[:],
                                 start=True, stop=True)
                Bc = work_s.tile([128, 128], f32r, name="Bc")
                nc.vector.tensor_copy(Bc[:], bc_ps[:])

                # transpose unnormalized ext probs -> Pt (E-major)
                Pt = work_a.tile([128, NE * 128], f32r, name="Pt")
                for ec in range(NE):
                    pst = ps_tr.tile([128, 128], f32r, name="pst")
                    nc.tensor.transpose(pst[:],
                                        pe[:, ec * 128:(ec + 1) * 128],
                                        ident[:])
                    nc.vector.tensor_copy(Pt[:, ec * 128:(ec + 1) * 128],
                                          pst[:])

                # ctx = Pt.T @ Vx + (xT*p_self).T @ Wv + p_self x bv,
                # all in one PSUM accumulation group per 512-col chunk
                psc = [ps_c.tile([128, WH], f32, name="psc")
                       for _ in range(NH)]
                for ec in range(NE):
                    lhsT = Pt[:, ec * 128:(ec + 1) * 128]
                    for n in range(NH):
                        nc.tensor.matmul(
                            psc[n][:], lhsT,
                            Vx[:, ec * H + n * WH: ec * H + (n + 1) * WH],
                            start=(ec == 0), stop=False)
                for kc in range(KC):
                    xs = work_s.tile([128, 128], f32r, name="xs")
                    nc.vector.tensor_tensor(
                        xs[:], xT[:, kc * T + m * 128: kc * T + (m + 1) * 128],
                        Bc[:], Alu.mult)
                    for n in range(NH):
                        nc.tensor.matmul(
                            psc[n][:], xs[:],
                            Wv[:, kc * H + n * WH: kc * H + (n + 1) * WH],
                            start=False, stop=(kc == KC - 1))

                # normalize on the way out
                out_sb = work_a.tile([128, H], f32, name="out_sb")
                for n in range(NH):
                    nc.vector.scalar_tensor_tensor(
                        out_sb[:, n * WH:(n + 1) * WH], psc[n][:], r[:],
                        bvb[:, n * WH:(n + 1) * WH], Alu.mult, Alu.add)
                nc.scalar.dma_start(out_d[m * 128:(m + 1) * 128, :], out_sb[:])


def _build_module(T, H, E, reps=1):
    from contextlib import ExitStack
    import concourse.tile as tile
    from concourse import bacc

    nc = bacc.Bacc(None)
    with ExitStack() as ctx:
        tc = ctx.enter_context(tile.TileContext(nc))
        _emit(nc, tc, ctx, T, H, E, reps)
    nc.finalize()
    return nc


# --------------------------------------------------------------------------
# host side
# --------------------------------------------------------------------------

def _shard_inputs(hidden_states, external_embeddings, Wq, bq, Wk, bk, Wv, bv):
    """Build the per-core input maps (host-side layout prep)."""
    hs = np.asarray(hidden_states, dtype=np.float32)
    ext = np.asarray(external_embeddings, dtype=np.float32)
    Wq = np.ascontiguousarray(np.asarray(Wq, dtype=np.float32))
    Wk = np.ascontiguousarray(np.asarray(Wk, dtype=np.float32))
    Wv = np.ascontiguousarray(np.asarray(Wv, dtype=np.float32))
    bq = np.asarray(bq, dtype=np.float32)
    bk = np.asarray(bk, dtype=np.float32)
    bv = np.asarray(bv, dtype=np.float32)

    KC = H // 128
    bqc = np.ascontiguousarray(bq.reshape(KC, 128).T)  # [128, KC]
    bkc = np.ascontiguousarray(bk.reshape(KC, 128).T)
    bvr = np.ascontiguousarray(bv.reshape(1, H))

    flat = hs.reshape(B * S, H)
    in_maps = []
    for c in range(NCORES):
        b = (c * T) // S
        xT = np.ascontiguousarray(flat[c * T:(c + 1) * T, :].T)  # [H, T]
        eT = np.ascontiguousarray(ext[b].T)                      # [H, E]
        in_maps.append({
            "xT": xT, "eT": eT,
            "Wq": Wq, "Wk": Wk, "Wv": Wv,
            "bqc": bqc, "bkc": bkc, "bvr": bvr,
            "onesl": _ONESL, "onesc": _ONESC, "eye": _EYE,
        })
    return in_maps


def kernel(hidden_states, external_embeddings, Wq, bq, Wk, bk, Wv, bv):
    from concourse.bass_utils import run_bass_kernel_spmd

    key = "main"
    if key not in _RUNNER_CACHE:
        _RUNNER_CACHE[key] = _build_module(T, H, E)
    nc = _RUNNER_CACHE[key]

    in_maps = _shard_inputs(hidden_states, external_embeddings,
                            Wq, bq, Wk, bk, Wv, bv)
    res = run_bass_kernel_spmd(nc, in_maps, list(range(NCORES)))
    out = np.concatenate([res.results[c]["out"] for c in range(NCORES)],
                         axis=0)
    return out.reshape(B, S, H)



# revision 45
# speedup vs baseline: 3.3223x; 3.3223x over previous
"""Trainium2 Bass kernel for ExternalEmbeddingSelfAttention.

Computation (per batch b):
    q     = hs @ Wq + bq           [S,H]
    k_tok = hs @ Wk + bk           [S,H]
    v_tok = hs @ Wv + bv           [S,H]
    k_ext = ext @ Wk + bk          [E,H]
    v_ext = ext @ Wv + bv          [E,H]
    s_self[t] = q[t] . k_tok[t]                (per-token self score)
    s_ext = q @ k_ext^T            [S,E]
    probs = softmax([s_self, s_ext], axis=-1)  (no 1/sqrt(d) scaling)
    out   = probs[:,0:1]*v_tok + probs[:,1:] @ v_ext

Sharding: 8 cores, each takes 1024 contiguous tokens of the flattened
(B*S, H) token axis (core i -> batch i//2, S-half i%2).  Each core also
computes its batch's external projections (duplicated across the 2 cores
sharing a batch).

Device schedule (per core, T=1024):  the DMA device is a single shared
332 GB/s resource, so every input DMA is issued on ONE queue (sync) in
exactly first-needed-first order; weights stream in natural row-chunk
layout (contiguous).  Phases:
  QA  (ho 0..3, kc-outer, full T): needs xT[kc]+WqA[kc] just-in-time;
      first matmul starts ~6us after launch instead of ~19us.
  QB  (ho 4..7): xT resident, WqB[kc] streams; Wk/eT prefetch behind.
  K_ext (kc-outer): Wk[kc]+eT[kc] mostly prefetched; KxT = Wk^T@eT+bk.
  K_tok+s_self (ho-outer): Wk/xT resident; d=(k+bk)*q accumulated on
      DVE into dsum, single ones-matmul -> s_self (saves PE work).
  V_ext (kc-outer): Vx = eT^T @ Wv (+0), stored bf16.
  Attention per 128-token tile: s_ext = QT.T@KxT (f32r); softmax with
      self column folded in; unnormalized probs PE-transposed; ctx PSUM
      accumulates ext context (Pt.T@Vx) and the self term
      ((xT*p_self_bcast).T@Wv); readout applies 1/Z and bv, split per
      512-col chunk so the last store overlaps the last readout.

Wv/Vx are bf16 (same PE rate, half DMA/SBUF; value-path precision is
ample).  Score pipeline stays f32r (FP22) end to end.
"""

import numpy as np
import ml_dtypes

B, S, E, H = 4, 2048, 512, 1024
NCORES = 8
T = (B * S) // NCORES  # tokens per core = 1024

_RUNNER_CACHE = {}

_ONESL = np.ones((1, 128), dtype=ml_dtypes.bfloat16)
_ONESC = np.ones((128, 2), dtype=np.float32)
_EYEB = np.eye(128, dtype=ml_dtypes.bfloat16)


# --------------------------------------------------------------------------
# device kernel emission
# --------------------------------------------------------------------------

def _emit(nc, tc, ctx, T, H, E, reps=1):
    import contextlib
    import concourse.mybir as mybir

    f32 = mybir.dt.float32
    f32r = mybir.dt.float32r
    bf16 = mybir.dt.bfloat16
    f8 = mybir.dt.float8e4
    DR = mybir.MatmulPerfMode.DoubleRow
    Alu = mybir.AluOpType
    Act = mybir.ActivationFunctionType
    X = mybir.AxisListType.X

    KC = H // 128          # contraction chunks over h_in; also h_out tiles
    NT = T // 128          # token tiles
    NE = E // 128          # ext tiles
    WH = min(512, H)       # h_out free-dim chunk
    NH = H // WH
    WT = min(512, T)       # token free-dim chunk
    NTW = T // WT
    assert E <= 512 and NTW == 2 and NH == 2 and KC == 8

    xT_d = nc.declare_dram_parameter("xT", [H, T], f32, isOutput=False)
    eT_d = nc.declare_dram_parameter("eT", [H, E], f32, isOutput=False)
    eT8h_d = nc.declare_dram_parameter("eT8h", [H // 2, 2 * E], f8,
                                       isOutput=False)
    eT8l_d = nc.declare_dram_parameter("eT8l", [H // 2, 2 * E], f8,
                                       isOutput=False)
    xT8h_d = nc.declare_dram_parameter("xT8h", [H // 2, 2 * T], f8,
                                       isOutput=False)
    xT8l_d = nc.declare_dram_parameter("xT8l", [H // 2, 2 * T], f8,
                                       isOutput=False)
    wq_d = nc.declare_dram_parameter("Wq", [H, H], f32, isOutput=False)
    wk_d = nc.declare_dram_parameter("Wk", [H, H], f32, isOutput=False)
    wv8h_d = nc.declare_dram_parameter("Wv8h", [H // 2, 2 * H], f8,
                                       isOutput=False)
    wv8l_d = nc.declare_dram_parameter("Wv8l", [H // 2, 2 * H], f8,
                                       isOutput=False)
    bqc_d = nc.declare_dram_parameter("bqc", [128, KC], f32, isOutput=False)
    bkc_d = nc.declare_dram_parameter("bkc", [128, KC], f32, isOutput=False)
    bvr_d = nc.declare_dram_parameter("bvr", [1, H], bf16, isOutput=False)
    onesl_d = nc.declare_dram_parameter("onesl", [1, 128], bf16, isOutput=False)
    onesc_d = nc.declare_dram_parameter("onesc", [128, 2], f32, isOutput=False)
    eye_d = nc.declare_dram_parameter("eye", [128, 128], bf16, isOutput=False)
    out_d = nc.declare_dram_parameter("out", [T, H], f32, isOutput=True)

    cp = ctx.enter_context(tc.tile_pool(name="cp", bufs=1))
    small = ctx.enter_context(tc.tile_pool(name="small", bufs=3))

    # ---- persistent SBUF tensors -----------------------------------------
    QT = cp.tile([128, KC * T], f32r, name="QT")    # Q^T hidden-major
    KxT = cp.tile([128, KC * E], f32r, name="KxT")  # K_ext^T hidden-major
    Vx8h = cp.tile([128, NE * H], f8, name="Vx8h")  # V_ext E-major fp8 hi
    Vx8l = cp.tile([128, NE * H], f8, name="Vx8l")  # and lo residual
    xT = cp.tile([128, KC * T], f32r, name="xT")    # chunk kc: cols [kc*T,+T)
    Wk = cp.tile([128, KC * H], f32r, name="Wk")    # Wk rows, chunk-major
    eT = cp.tile([128, KC * E], f32r, name="eT")
    NP = KC // 2  # DoubleRow kc-pairs
    Wv8h = cp.tile([128, NP * 2 * H], f8, name="Wv8h")  # 32*Wv hi/lo fp8
    Wv8l = cp.tile([128, NP * 2 * H], f8, name="Wv8l")
    eT8h = cp.tile([128, NP * 2 * E], f8, name="eT8h")
    eT8l = cp.tile([128, NP * 2 * E], f8, name="eT8l")
    xT8h = cp.tile([128, NP * 2 * T], f8, name="xT8h")
    xT8l = cp.tile([128, NP * 2 * T], f8, name="xT8l")
    dsum = cp.tile([128, T], f32r, name="dsum")     # per-chunk q*k partials
    ss_col = cp.tile([128, NT], f32, name="ss_col")
    bqc = cp.tile([128, KC], f32, name="bqc")
    bkc = cp.tile([128, KC], f32, name="bkc")
    bvr = cp.tile([1, H], bf16, name="bvr")
    bvb = cp.tile([128, H], bf16, name="bvb")
    ones_c = cp.tile([128, 2], f32r, name="ones_c")
    identb = cp.tile([128, 128], bf16, name="identb")

    loop_cm = tc.For_i(0, reps, 1) if reps > 1 else contextlib.nullcontext()
    with loop_cm:
      with tc.tile_pool(name="mono", bufs=8, space="PSUM") as mono, \
           tc.tile_pool(name="work_k", bufs=1) as work_k:
        with tc.tile_pool(name="wq", bufs=4) as wq_pool:
            wqa = [wq_pool.tile([128, WT], f32r, name="wqa") for _ in range(KC)]
            wqb = [wq_pool.tile([128, WT], f32r, name="wqb") for _ in range(KC)]

            # ---- the ordered input stream (single queue = device order) --
            for kc in range(KC):
                nc.sync.dma_start(xT[:, kc * T:(kc + 1) * T],
                                  xT_d[kc * 128:(kc + 1) * 128, :].bitcast(f32r))
                nc.sync.dma_start(
                    wqa[kc][:],
                    wq_d[kc * 128:(kc + 1) * 128, 0:WT].bitcast(f32r))
                if kc == 2:
                    nc.sync.dma_start(bqc[:], bqc_d[:])
                    nc.sync.dma_start(bkc[:], bkc_d[:])
            for kc in range(KC):
                nc.sync.dma_start(
                    wqb[kc][:],
                    wq_d[kc * 128:(kc + 1) * 128, WT:H].bitcast(f32r))
            for kc in range(KC):
                nc.sync.dma_start(Wk[:, kc * H:(kc + 1) * H],
                                  wk_d[kc * 128:(kc + 1) * 128, :].bitcast(f32r))
                nc.sync.dma_start(eT[:, kc * E:(kc + 1) * E],
                                  eT_d[kc * 128:(kc + 1) * 128, :].bitcast(f32r))
            nc.sync.dma_start(ones_c[:], onesc_d[:].bitcast(f32r))
            nc.sync.dma_start(bvr[:], bvr_d[:])
            nc.sync.dma_start(ones_lb[:], onesl_d[:])
            nc.sync.dma_start(identb[:], eye_d[:])
            for pr in range(NP):
                nc.sync.dma_start(Wv8h[:, pr * 2 * H:(pr + 1) * 2 * H],
                                  wv8h_d[pr * 128:(pr + 1) * 128, :])
                nc.sync.dma_start(Wv8l[:, pr * 2 * H:(pr + 1) * 2 * H],
                                  wv8l_d[pr * 128:(pr + 1) * 128, :])
            for pr in range(NP):
                nc.sync.dma_start(eT8h[:, pr * 2 * E:(pr + 1) * 2 * E],
                                  eT8h_d[pr * 128:(pr + 1) * 128, :])
                nc.sync.dma_start(eT8l[:, pr * 2 * E:(pr + 1) * 2 * E],
                                  eT8l_d[pr * 128:(pr + 1) * 128, :])
            for pr in range(NP):
                nc.sync.dma_start(xT8h[:, pr * 2 * T:(pr + 1) * 2 * T],
                                  xT8h_d[pr * 128:(pr + 1) * 128, :])
                nc.sync.dma_start(xT8l[:, pr * 2 * T:(pr + 1) * 2 * T],
                                  xT8l_d[pr * 128:(pr + 1) * 128, :])


            # preload the Exp activation table off the critical path
            dummy = small.tile([1, 2], f32, name="dummy")
            nc.scalar.memzero(dummy[:])
            nc.scalar.activation(dummy[:], dummy[:], Act.Exp, bias=0.0,
                                 scale=1.0)

            # ---- QA / QB: QT = Wq^T @ xT (+bq), kc-outer, half-ho --------
            # All pre-attention phases share the mono PSUM pool: bank
            # handoff is a per-bank WAR dep (phase N+1's start=True matmul
            # on a bank waits only that bank's readout), not a pool barrier.
            for half, wqs in ((0, wqa), (1, wqb)):
                psq = [mono.tile([128, WT], f32, name="psq", tag="acc")
                       for _ in range(8)]
                for kc in range(KC):
                    for ho4 in range(4):
                        for n in range(NTW):
                            nc.tensor.matmul(
                                psq[ho4 * NTW + n][:],
                                wqs[kc][:, ho4 * 128:(ho4 + 1) * 128],
                                xT[:, kc * T + n * WT: kc * T + (n + 1) * WT],
                                start=(kc == 0), stop=(kc == KC - 1))
                for ho4 in range(4):
                    ho = half * 4 + ho4
                    for n in range(NTW):
                        nc.vector.tensor_scalar_add(
                            QT[:, ho * T + n * WT: ho * T + (n + 1) * WT],
                            psq[ho4 * NTW + n][:], bqc[:, ho:ho + 1])

        # ---- K_ext: KxT = Wk^T @ eT (+bk), kc-outer, 2 sub-phases --------
        for hblk in range(2):
            pske = [mono.tile([128, E], f32, name="pske", tag="acc") for _ in range(4)]
            for kc in range(KC):
                for h4 in range(4):
                    ho = hblk * 4 + h4
                    nc.tensor.matmul(
                        pske[h4][:],
                        Wk[:, kc * H + ho * 128: kc * H + (ho + 1) * 128],
                        eT[:, kc * E:(kc + 1) * E],
                        start=(kc == 0), stop=(kc == KC - 1))
            for h4 in range(4):
                ho = hblk * 4 + h4
                nc.vector.tensor_scalar_add(KxT[:, ho * E:(ho + 1) * E],
                                            pske[h4][:], bkc[:, ho:ho + 1])

        # ---- K_tok + s_self (ho-outer; Wk/xT resident) -------------------
        # bvb = bv broadcast to all partitions (built here, PSUM has slack)
        for n in range(NH):
            pbv = mono.tile([128, WH], f32, name="pbv", tag="acc")
            nc.tensor.matmul(pbv[:], ones_lb[:],
                             bvr[:, n * WH:(n + 1) * WH],
                             start=True, stop=True)
            nc.vector.tensor_copy(bvb[:, n * WH:(n + 1) * WH], pbv[:])
        for ho in range(KC):
            pskt = [mono.tile([128, WT], f32, name="pskt", tag="acc")
                    for _ in range(NTW)]
            for kc in range(KC):
                for n in range(NTW):
                    nc.tensor.matmul(
                        pskt[n][:],
                        Wk[:, kc * H + ho * 128: kc * H + (ho + 1) * 128],
                        xT[:, kc * T + n * WT: kc * T + (n + 1) * WT],
                        start=(kc == 0), stop=(kc == KC - 1))
            # d = (k_tok^T + bk) * QT, straight from PSUM; accumulate
            # into dsum on DVE (cheaper than per-ho PE ones-matmuls)
            if ho == 0:
                for n in range(NTW):
                    nc.vector.scalar_tensor_tensor(
                        dsum[:, n * WT:(n + 1) * WT], pskt[n][:],
                        bkc[:, ho:ho + 1],
                        QT[:, ho * T + n * WT: ho * T + (n + 1) * WT],
                        Alu.add, Alu.mult)
            else:
                for n in range(NTW):
                    d = work_k.tile([128, WT], f32r, name="d")
                    nc.vector.scalar_tensor_tensor(
                        d[:], pskt[n][:], bkc[:, ho:ho + 1],
                        QT[:, ho * T + n * WT: ho * T + (n + 1) * WT],
                        Alu.add, Alu.mult)
                    nc.vector.tensor_tensor(
                        dsum[:, n * WT:(n + 1) * WT],
                        dsum[:, n * WT:(n + 1) * WT], d[:], Alu.add)

        # ---- V_ext: Vx = eT^T @ Wv via fp8 DoubleRow; Wv is pre-scaled
        # x32 on the host so its hi/lo fp8 split avoids e4m3 subnormals,
        # and the 1/32 is applied at readout.  3 cross terms; the lo*lo
        # term (~0.4%) is dropped. ----------------------------------------
        def pair2(t, pr, width, lo, hi):
            return t[:, pr * 2 * width:(pr + 1) * 2 * width].rearrange(
                "p (two w) -> p two w", two=2)[:, :, lo:hi]

        VTERMS = ((eT8h, Wv8h), (eT8h, Wv8l), (eT8l, Wv8h))
        for eblk in range(2):
            psv = [mono.tile([128, WH], f32, name="psv", tag="acc")
                   for _ in range(4)]
            for pr in range(NP):
                for ti, (lt, rt) in enumerate(VTERMS):
                    for e2 in range(2):
                        eo = eblk * 2 + e2
                        for n in range(NH):
                            nc.tensor.matmul(
                                psv[e2 * NH + n][:],
                                pair2(lt, pr, E, eo * 128, (eo + 1) * 128),
                                pair2(rt, pr, H, n * WH, (n + 1) * WH),
                                start=(pr == 0 and ti == 0),
                                stop=(pr == NP - 1 and ti == len(VTERMS) - 1),
                                perf_mode=DR)
            for e2 in range(2):
                eo = eblk * 2 + e2
                for n in range(NH):
                    ph = Vx8h[:, eo * H + n * WH: eo * H + (n + 1) * WH]
                    pl = Vx8l[:, eo * H + n * WH: eo * H + (n + 1) * WH]
                    nc.scalar.activation(ph, psv[e2 * NH + n][:], Act.Copy,
                                         bias=0.0, scale=1.0 / 32)
                    nc.vector.scalar_tensor_tensor(
                        pl, psv[e2 * NH + n][:], 1.0 / 32, ph,
                        Alu.mult, Alu.subtract)

        # s_self per token tile: column-sum dsum via tiny matmuls
        # (lhsT = dsum slice, rhs = ones column) -> [128 tokens, 1]
        for m in range(NT):
            ssp = mono.tile([128, 2], f32, name="ssp", tag="acc")
            nc.tensor.matmul(ssp[:], dsum[:, m * 128:(m + 1) * 128],
                             ones_c[:], start=True, stop=True)
            nc.vector.tensor_copy(ss_col[:, m:m + 1], ssp[:, 0:1])

      # ---- attention per token tile ------------------------------------
      # v_tok runs UNSCALED (xTb @ Wv, no softmax dependency) in its own
      # PSUM groups, filling the PE while the softmax chain computes; the
      # p_self scaling is applied per-partition at readout.  This removes
      # the p_self broadcast (transpose+matmul) chain entirely.
      with tc.tile_pool(name="ps_att", bufs=2, space="PSUM") as ps_att, \
           tc.tile_pool(name="ps_tr", bufs=2, space="PSUM") as ps_tr, \
           tc.tile_pool(name="ps_cu", bufs=1, space="PSUM") as ps_cu, \
           tc.tile_pool(name="work_a", bufs=2) as work_a, \
           tc.tile_pool(name="pt8", bufs=2) as pt8_pool:
        for m in range(NT):
            last = m == NT - 1
            # s_ext = Q^T.T @ K_ext^T  -> [128 tokens, E]  (f32r)
            ps_s = ps_att.tile([128, E], f32, name="ps_s")
            for kc in range(KC):
                nc.tensor.matmul(
                    ps_s[:],
                    QT[:, kc * T + m * 128: kc * T + (m + 1) * 128],
                    KxT[:, kc * E:(kc + 1) * E],
                    start=(kc == 0), stop=(kc == KC - 1))

            nmx = small.tile([128, 1], f32, name="nmx")
            nc.vector.tensor_reduce(nmx[:], ps_s[:], axis=X, op=Alu.max,
                                    negate=True)
            # nmx2 = min(-ss, nmx) = -max(s_self, max(s_ext))
            nmx2 = small.tile([128, 1], f32, name="nmx2")
            nc.vector.scalar_tensor_tensor(
                nmx2[:], ss_col[:, m:m + 1], -1.0, nmx[:],
                Alu.mult, Alu.min)
            # probs are computed x256 (bias += ln 256) so their fp8 hi/lo
            # split stays clear of e4m3 subnormals; 1/256 folds into r.
            nmx2b = small.tile([128, 1], f32, name="nmx2b")
            nc.vector.tensor_scalar_add(nmx2b[:], nmx2[:], 4.85203026)

            pe = work_a.tile([128, E], bf16, name="pe")
            Ze = small.tile([128, 1], f32, name="Ze")
            nc.scalar.activation(pe[:], ps_s[:], Act.Exp, bias=nmx2b[:],
                                 scale=1.0, accum_out=Ze[:])
            p_self = small.tile([128, 1], f32, name="p_self")
            nc.scalar.activation(p_self[:], ss_col[:, m:m + 1],
                                 Act.Exp, bias=nmx2[:], scale=1.0)
            Zt = small.tile([128, 1], f32, name="Zt")
            nc.vector.scalar_tensor_tensor(Zt[:], Ze[:], 1.0 / 128,
                                           p_self[:], Alu.mult, Alu.add)
            r = small.tile([128, 1], f32, name="r")
            nc.vector.reciprocal(r[:], Zt[:])
            pr = small.tile([128, 1], f32, name="pr")
            nc.vector.scalar_tensor_tensor(pr[:], p_self[:], 1.0 / 32,
                                           r[:], Alu.mult, Alu.mult)
            r256 = small.tile([128, 1], f32, name="r256")
            nc.vector.tensor_scalar_mul(r256[:], r[:], 1.0 / 128)

            # unscaled v_tok: psu[n] = (xTb slice).T @ Wv — independent of
            # the softmax, keeps the PE busy during the chain above
            psu = [ps_cu.tile([128, WH], f32, name=f"psu{n}", tag=f"u{n}")
                   for n in range(NH)]
            UTERMS = ((xT8h, Wv8h), (xT8h, Wv8l), (xT8l, Wv8h))
            for pr8 in range(NP):
                for ti, (lt, rt) in enumerate(UTERMS):
                    lhsT = pair2(lt, pr8, T, m * 128, (m + 1) * 128)
                    for n in range(NH):
                        nc.tensor.matmul(
                            psu[n][:], lhsT,
                            pair2(rt, pr8, H, n * WH, (n + 1) * WH),
                            start=(pr8 == 0 and ti == 0),
                            stop=(pr8 == NP - 1 and ti == len(UTERMS) - 1),
                            perf_mode=DR)

            # transpose unnormalized ext probs -> Pt (E-major, bf16),
            # then split hi/lo fp8 (copy on DVE, residual on Pool)
            Pt = work_a.tile([128, NE * 128], bf16, name="Pt")
            for ec in range(NE):
                pst = ps_tr.tile([128, 128], bf16, name="pst")
                nc.tensor.transpose(pst[:],
                                    pe[:, ec * 128:(ec + 1) * 128],
                                    identb[:])
                nc.vector.tensor_copy(Pt[:, ec * 128:(ec + 1) * 128],
                                      pst[:])
            Pt8h = pt8_pool.tile([128, NE * 128], f8, name="Pt8h")
            Pt8l = pt8_pool.tile([128, NE * 128], f8, name="Pt8l")
            nc.vector.tensor_copy(Pt8h[:], Pt[:])
            nc.gpsimd.tensor_tensor(Pt8l[:], Pt[:], Pt8h[:], Alu.subtract)

            # ctx_ext = Pt.T @ Vx
            psc = [ps_cu.tile([128, WH], f32, name=f"psc{n}", tag=f"c{n}")
                   for n in range(NH)]
            CTERMS = ((Pt8h, Vx8h), (Pt8h, Vx8l), (Pt8l, Vx8h))
            NEP = NE // 2  # ec-pairs

            def ctx_mm(n, ep, ti):
                lt, rt = CTERMS[ti]
                nc.tensor.matmul(
                    psc[n][:],
                    lt[:].rearrange("p (ep two e) -> p ep two e",
                                    ep=NEP, two=2)[:, ep],
                    rt[:, 2 * ep * H:(2 * ep + 2) * H].rearrange(
                        "p (two h) -> p two h",
                        two=2)[:, :, n * WH:(n + 1) * WH],
                    start=(ep == 0 and ti == 0),
                    stop=(ep == NEP - 1 and ti == len(CTERMS) - 1),
                    perf_mode=DR)

            if not last:
                for ep in range(NEP):
                    for ti in range(len(CTERMS)):
                        for n in range(NH):
                            ctx_mm(n, ep, ti)
            else:
                # finish chunk 1 first so its readout+store overlaps
                # chunk 0's matmuls and the drain is one chunk shorter
                for n in (1, 0):
                    for ep in range(NEP):
                        for ti in range(len(CTERMS)):
                            ctx_mm(n, ep, ti)

            # out = r*ctx_ext + (p_self*r)*v_tok + bvb, stored in strips
            WS = 256 if last else WH
            for n in ((1, 0) if last else range(NH)):
                osb = work_a.tile([128, WH], f32, name="osb")
                for s2 in range(WH // WS):
                    lo = s2 * WS
                    # psu finishes well before psc: fold its term early on
                    # the idle Pool engine; the final stt is one DVE op
                    # after ctx_ext stops.
                    nc.vector.scalar_tensor_tensor(
                        osb[:, lo:lo + WS], psu[n][:, lo:lo + WS],
                        pr[:], bvb[:, n * WH + lo:n * WH + lo + WS],
                        Alu.mult, Alu.add)
                    nc.vector.scalar_tensor_tensor(
                        osb[:, lo:lo + WS], psc[n][:, lo:lo + WS],
                        r256[:], osb[:, lo:lo + WS],
                        Alu.mult, Alu.add)
                    nc.scalar.dma_start(
                        out_d[m * 128:(m + 1) * 128,
                              n * WH + lo:n * WH + lo + WS],
                        osb[:, lo:lo + WS])


def _build_module(T, H, E, reps=1):
    from contextlib import ExitStack
    import concourse.tile as tile
    from concourse import bacc

    nc = bacc.Bacc(None)
    with ExitStack() as ctx:
        tc = ctx.enter_context(tile.TileContext(nc))
        _emit(nc, tc, ctx, T, H, E, reps)
    nc.finalize()
    return nc


# --------------------------------------------------------------------------
# host side
# --------------------------------------------------------------------------

def _shard_inputs(hidden_states, external_embeddings, Wq, bq, Wk, bk, Wv, bv):
    """Build the per-core input maps (host-side layout prep)."""
    hs = np.asarray(hidden_states, dtype=np.float32)
    ext = np.asarray(external_embeddings, dtype=np.float32)
    Wq = np.ascontiguousarray(np.asarray(Wq, dtype=np.float32))
    Wk = np.ascontiguousarray(np.asarray(Wk, dtype=np.float32))
    Wv = np.asarray(Wv, dtype=np.float32)
    bq = np.asarray(bq, dtype=np.float32)
    bk = np.asarray(bk, dtype=np.float32)
    bv = np.asarray(bv, dtype=np.float32)

    f8 = ml_dtypes.float8_e4m3

    def fp8_pairs(a):
        """[H, N] f32 -> hi/lo fp8 arrays [H//2, 2*N] in DoubleRow
        kc-pair layout: row pr*128+p holds chunks (2pr, 2pr+1)."""
        Hd, N = a.shape
        hi = a.astype(f8)
        lo = (a - hi.astype(np.float32)).astype(f8)
        out = []
        for arr in (hi, lo):
            v = arr.reshape(Hd // 256, 2, 128, N).transpose(0, 2, 1, 3)
            out.append(np.ascontiguousarray(v.reshape(Hd // 2, 2 * N)))
        return out

    Wv8h, Wv8l = fp8_pairs(Wv * 32.0)

    KC = H // 128
    bqc = np.ascontiguousarray(bq.reshape(KC, 128).T)  # [128, KC]
    bkc = np.ascontiguousarray(bk.reshape(KC, 128).T)
    bvr = np.ascontiguousarray(bv.reshape(1, H).astype(ml_dtypes.bfloat16))

    flat = hs.reshape(B * S, H)
    in_maps = []
    _ET8 = {}
    for c in range(NCORES):
        b = (c * T) // S
        xT = np.ascontiguousarray(flat[c * T:(c + 1) * T, :].T)  # [H, T]
        eT = np.ascontiguousarray(ext[b].T)                      # [H, E]
        eT8h, eT8l = _ET8.setdefault(b, fp8_pairs(eT))
        xT8h, xT8l = fp8_pairs(xT)
        in_maps.append({
            "xT": xT, "xT8h": xT8h, "xT8l": xT8l,
            "eT": eT, "eT8h": eT8h, "eT8l": eT8l,
            "Wq": Wq, "Wk": Wk, "Wv8h": Wv8h, "Wv8l": Wv8l,
            "bqc": bqc, "bkc": bkc, "bvr": bvr,
            "onesl": _ONESL, "onesc": _ONESC, "eye": _EYEB,
        })
    return in_maps


def kernel(hidden_states, external_embeddings, Wq, bq, Wk, bk, Wv, bv):
    from concourse.bass_utils import run_bass_kernel_spmd

    key = "main"
    if key not in _RUNNER_CACHE:
        _RUNNER_CACHE[key] = _build_module(T, H, E)
    nc = _RUNNER_CACHE[key]

    in_maps = _shard_inputs(hidden_states, external_embeddings,
                            Wq, bq, Wk, bk, Wv, bv)
    res = run_bass_kernel_spmd(nc, in_maps, list(range(NCORES)))
    out = np.concatenate([res.results[c]["out"] for c in range(NCORES)],
                         axis=0)
    return out.reshape(B, S, H)


# revision 50
# speedup vs baseline: 3.3737x; 1.0155x over previous
"""Trainium2 Bass kernel for ExternalEmbeddingSelfAttention.

Computation (per batch b):
    q     = hs @ Wq + bq           [S,H]
    k_tok = hs @ Wk + bk           [S,H]
    v_tok = hs @ Wv + bv           [S,H]
    k_ext = ext @ Wk + bk          [E,H]
    v_ext = ext @ Wv + bv          [E,H]
    s_self[t] = q[t] . k_tok[t]                (per-token self score)
    s_ext = q @ k_ext^T            [S,E]
    probs = softmax([s_self, s_ext], axis=-1)  (no 1/sqrt(d) scaling)
    out   = probs[:,0:1]*v_tok + probs[:,1:] @ v_ext

Sharding: 8 cores, each takes 1024 contiguous tokens of the flattened
(B*S, H) token axis (core i -> batch i//2, S-half i%2).  Each core also
computes its batch's external projections (duplicated across the 2 cores
sharing a batch).

Device schedule (per core, T=1024):  the DMA device is a single shared
332 GB/s resource, so every input DMA is issued on ONE queue (sync) in
exactly first-needed-first order; weights stream in natural row-chunk
layout (contiguous).  Phases:
  QA  (ho 0..3, kc-outer, full T): needs xT[kc]+WqA[kc] just-in-time;
      first matmul starts ~6us after launch instead of ~19us.
  QB  (ho 4..7): xT resident, WqB[kc] streams; Wk/eT prefetch behind.
  K_ext (kc-outer): Wk[kc]+eT[kc] mostly prefetched; KxT = Wk^T@eT+bk.
  K_tok+s_self (ho-outer): Wk/xT resident; d=(k+bk)*q accumulated on
      DVE into dsum, single ones-matmul -> s_self (saves PE work).
  V_ext (kc-outer): Vx = eT^T @ Wv (+0), stored bf16.
  Attention per 128-token tile: s_ext = QT.T@KxT (f32r); softmax with
      self column folded in; unnormalized probs PE-transposed; ctx PSUM
      accumulates ext context (Pt.T@Vx) and the self term
      ((xT*p_self_bcast).T@Wv); readout applies 1/Z and bv, split per
      512-col chunk so the last store overlaps the last readout.

Wv/Vx are bf16 (same PE rate, half DMA/SBUF; value-path precision is
ample).  Score pipeline stays f32r (FP22) end to end.
"""

import numpy as np
import ml_dtypes

B, S, E, H = 4, 2048, 512, 1024
NCORES = 8
T = (B * S) // NCORES  # tokens per core = 1024

_RUNNER_CACHE = {}

_ONESL = np.ones((1, 128), dtype=ml_dtypes.bfloat16)
_ONESC = np.ones((128, 2), dtype=np.float32)
_EYEB = np.eye(128, dtype=ml_dtypes.bfloat16)


# --------------------------------------------------------------------------
# device kernel emission
# --------------------------------------------------------------------------

def _emit(nc, tc, ctx, T, H, E, reps=1):
    import contextlib
    import concourse.mybir as mybir

    f32 = mybir.dt.float32
    f32r = mybir.dt.float32r
    bf16 = mybir.dt.bfloat16
    f8 = mybir.dt.float8e4
    DR = mybir.MatmulPerfMode.DoubleRow
    Alu = mybir.AluOpType
    Act = mybir.ActivationFunctionType
    X = mybir.AxisListType.X

    KC = H // 128          # contraction chunks over h_in; also h_out tiles
    NT = T // 128          # token tiles
    NE = E // 128          # ext tiles
    WH = min(512, H)       # h_out free-dim chunk
    NH = H // WH
    WT = min(512, T)       # token free-dim chunk
    NTW = T // WT
    assert E <= 512 and NTW == 2 and NH == 2 and KC == 8

    xT_d = nc.declare_dram_parameter("xT", [H, T], f32, isOutput=False)
    eT_d = nc.declare_dram_parameter("eT", [H, E], f32, isOutput=False)
    eT8h_d = nc.declare_dram_parameter("eT8h", [H // 2, 2 * E], f8,
                                       isOutput=False)
    eT8l_d = nc.declare_dram_parameter("eT8l", [H // 2, 2 * E], f8,
                                       isOutput=False)
    xT8h_d = nc.declare_dram_parameter("xT8h", [H // 2, 2 * T], f8,
                                       isOutput=False)
    xT8l_d = nc.declare_dram_parameter("xT8l", [H // 2, 2 * T], f8,
                                       isOutput=False)
    wq_d = nc.declare_dram_parameter("Wq", [H, H], f32, isOutput=False)
    wk_d = nc.declare_dram_parameter("Wk", [H, H], f32, isOutput=False)
    wv8h_d = nc.declare_dram_parameter("Wv8h", [H // 2, 2 * H], f8,
                                       isOutput=False)
    wv8l_d = nc.declare_dram_parameter("Wv8l", [H // 2, 2 * H], f8,
                                       isOutput=False)
    bqc_d = nc.declare_dram_parameter("bqc", [128, KC], f32, isOutput=False)
    bkc_d = nc.declare_dram_parameter("bkc", [128, KC], f32, isOutput=False)
    bvr_d = nc.declare_dram_parameter("bvr", [1, H], bf16, isOutput=False)
    onesl_d = nc.declare_dram_parameter("onesl", [1, 128], bf16, isOutput=False)
    onesc_d = nc.declare_dram_parameter("onesc", [128, 2], f32, isOutput=False)
    eye_d = nc.declare_dram_parameter("eye", [128, 128], bf16, isOutput=False)
    out_d = nc.declare_dram_parameter("out", [T, H], f32, isOutput=True)

    cp = ctx.enter_context(tc.tile_pool(name="cp", bufs=1))
    small = ctx.enter_context(tc.tile_pool(name="small", bufs=3))

    # ---- persistent SBUF tensors -----------------------------------------
    QT = cp.tile([128, KC * T], f32r, name="QT")    # Q^T hidden-major
    KxT = cp.tile([128, KC * E], f32r, name="KxT")  # K_ext^T hidden-major
    Vx8h = cp.tile([128, NE * H], f8, name="Vx8h")  # V_ext E-major fp8 hi
    Vx8l = cp.tile([128, NE * H], f8, name="Vx8l")  # and lo residual
    xT = cp.tile([128, KC * T], f32r, name="xT")    # chunk kc: cols [kc*T,+T)
    Wk = cp.tile([128, KC * H], f32r, name="Wk")    # Wk rows, chunk-major
    eT = cp.tile([128, KC * E], f32r, name="eT")
    NP = KC // 2  # DoubleRow kc-pairs
    Wv8h = cp.tile([128, NP * 2 * H], f8, name="Wv8h")  # 32*Wv hi/lo fp8
    Wv8l = cp.tile([128, NP * 2 * H], f8, name="Wv8l")
    eT8h = cp.tile([128, NP * 2 * E], f8, name="eT8h")
    eT8l = cp.tile([128, NP * 2 * E], f8, name="eT8l")
    xT8h = cp.tile([128, NP * 2 * T], f8, name="xT8h")
    xT8l = cp.tile([128, NP * 2 * T], f8, name="xT8l")
    dsum = cp.tile([128, T], f32r, name="dsum")     # per-chunk q*k partials
    ss_col = cp.tile([128, NT], f32, name="ss_col")
    bqc = cp.tile([128, KC], f32, name="bqc")
    bkc = cp.tile([128, KC], f32, name="bkc")
    bvr = cp.tile([1, H], bf16, name="bvr")
    bvb = cp.tile([128, H], bf16, name="bvb")
    ones_c = cp.tile([128, 2], f32r, name="ones_c")
    identb = cp.tile([128, 128], bf16, name="identb")

    loop_cm = tc.For_i(0, reps, 1) if reps > 1 else contextlib.nullcontext()
    with loop_cm:
      with tc.tile_pool(name="mono", bufs=8, space="PSUM") as mono, \
           tc.tile_pool(name="work_k", bufs=1) as work_k:
        with tc.tile_pool(name="wq", bufs=4) as wq_pool:
            wqa = [wq_pool.tile([128, WT], f32r, name="wqa") for _ in range(KC)]
            wqb = [wq_pool.tile([128, WT], f32r, name="wqb") for _ in range(KC)]

            # ---- the ordered input stream (single queue = device order) --
            for kc in range(KC):
                nc.sync.dma_start(xT[:, kc * T:(kc + 1) * T],
                                  xT_d[kc * 128:(kc + 1) * 128, :].bitcast(f32r))
                nc.sync.dma_start(
                    wqa[kc][:],
                    wq_d[kc * 128:(kc + 1) * 128, 0:WT].bitcast(f32r))
                if kc == 2:
                    nc.sync.dma_start(bqc[:], bqc_d[:])
                    nc.sync.dma_start(bkc[:], bkc_d[:])
            for kc in range(KC):
                nc.sync.dma_start(
                    wqb[kc][:],
                    wq_d[kc * 128:(kc + 1) * 128, WT:H].bitcast(f32r))
            for kc in range(KC):
                nc.sync.dma_start(Wk[:, kc * H:(kc + 1) * H],
                                  wk_d[kc * 128:(kc + 1) * 128, :].bitcast(f32r))
                nc.sync.dma_start(eT[:, kc * E:(kc + 1) * E],
                                  eT_d[kc * 128:(kc + 1) * 128, :].bitcast(f32r))
            nc.sync.dma_start(ones_c[:], onesc_d[:].bitcast(f32r))
            nc.sync.dma_start(bvr[:], bvr_d[:])
            nc.sync.dma_start(ones_lb[:], onesl_d[:])
            nc.sync.dma_start(identb[:], eye_d[:])
            for pr in range(NP):
                nc.sync.dma_start(Wv8h[:, pr * 2 * H:(pr + 1) * 2 * H],
                                  wv8h_d[pr * 128:(pr + 1) * 128, :])
                nc.sync.dma_start(Wv8l[:, pr * 2 * H:(pr + 1) * 2 * H],
                                  wv8l_d[pr * 128:(pr + 1) * 128, :])
            for pr in range(NP):
                nc.sync.dma_start(eT8h[:, pr * 2 * E:(pr + 1) * 2 * E],
                                  eT8h_d[pr * 128:(pr + 1) * 128, :])
                nc.sync.dma_start(eT8l[:, pr * 2 * E:(pr + 1) * 2 * E],
                                  eT8l_d[pr * 128:(pr + 1) * 128, :])
            for pr in range(NP):
                nc.sync.dma_start(xT8h[:, pr * 2 * T:(pr + 1) * 2 * T],
                                  xT8h_d[pr * 128:(pr + 1) * 128, :])
                nc.sync.dma_start(xT8l[:, pr * 2 * T:(pr + 1) * 2 * T],
                                  xT8l_d[pr * 128:(pr + 1) * 128, :])


            # preload the Exp activation table off the critical path
            dummy = small.tile([1, 2], f32, name="dummy")
            nc.scalar.memzero(dummy[:])
            nc.scalar.activation(dummy[:], dummy[:], Act.Exp, bias=0.0,
                                 scale=1.0)

            # ---- QA / QB: QT = Wq^T @ xT (+bq), kc-outer, half-ho --------
            # All pre-attention phases share the mono PSUM pool: bank
            # handoff is a per-bank WAR dep (phase N+1's start=True matmul
            # on a bank waits only that bank's readout), not a pool barrier.
            for half, wqs in ((0, wqa), (1, wqb)):
                psq = [mono.tile([128, WT], f32, name="psq", tag="acc")
                       for _ in range(8)]
                for kc in range(KC):
                    for ho4 in range(4):
                        for n in range(NTW):
                            nc.tensor.matmul(
                                psq[ho4 * NTW + n][:],
                                wqs[kc][:, ho4 * 128:(ho4 + 1) * 128],
                                xT[:, kc * T + n * WT: kc * T + (n + 1) * WT],
                                start=(kc == 0), stop=(kc == KC - 1))
                for ho4 in range(4):
                    ho = half * 4 + ho4
                    for n in range(NTW):
                        nc.vector.tensor_scalar_add(
                            QT[:, ho * T + n * WT: ho * T + (n + 1) * WT],
                            psq[ho4 * NTW + n][:], bqc[:, ho:ho + 1])

        # ---- K_ext: KxT = Wk^T @ eT (+bk), kc-outer, 2 sub-phases --------
        for hblk in range(2):
            pske = [mono.tile([128, E], f32, name="pske", tag="acc") for _ in range(4)]
            for kc in range(KC):
                for h4 in range(4):
                    ho = hblk * 4 + h4
                    nc.tensor.matmul(
                        pske[h4][:],
                        Wk[:, kc * H + ho * 128: kc * H + (ho + 1) * 128],
                        eT[:, kc * E:(kc + 1) * E],
                        start=(kc == 0), stop=(kc == KC - 1))
            for h4 in range(4):
                ho = hblk * 4 + h4
                nc.vector.tensor_scalar_add(KxT[:, ho * E:(ho + 1) * E],
                                            pske[h4][:], bkc[:, ho:ho + 1])

        # ---- K_tok + s_self (ho-outer; Wk/xT resident) -------------------
        # bvb = bv broadcast to all partitions (built here, PSUM has slack)
        for n in range(NH):
            pbv = mono.tile([128, WH], f32, name="pbv", tag="acc")
            nc.tensor.matmul(pbv[:], ones_lb[:],
                             bvr[:, n * WH:(n + 1) * WH],
                             start=True, stop=True)
            nc.vector.tensor_copy(bvb[:, n * WH:(n + 1) * WH], pbv[:])
        for ho in range(KC):
            pskt = [mono.tile([128, WT], f32, name="pskt", tag="acc")
                    for _ in range(NTW)]
            for kc in range(KC):
                for n in range(NTW):
                    nc.tensor.matmul(
                        pskt[n][:],
                        Wk[:, kc * H + ho * 128: kc * H + (ho + 1) * 128],
                        xT[:, kc * T + n * WT: kc * T + (n + 1) * WT],
                        start=(kc == 0), stop=(kc == KC - 1))
            # d = (k_tok^T + bk) * QT, straight from PSUM; accumulate
            # into dsum on DVE (cheaper than per-ho PE ones-matmuls)
            if ho == 0:
                for n in range(NTW):
                    nc.vector.scalar_tensor_tensor(
                        dsum[:, n * WT:(n + 1) * WT], pskt[n][:],
                        bkc[:, ho:ho + 1],
                        QT[:, ho * T + n * WT: ho * T + (n + 1) * WT],
                        Alu.add, Alu.mult)
            else:
                for n in range(NTW):
                    d = work_k.tile([128, WT], f32r, name="d")
                    nc.vector.scalar_tensor_tensor(
                        d[:], pskt[n][:], bkc[:, ho:ho + 1],
                        QT[:, ho * T + n * WT: ho * T + (n + 1) * WT],
                        Alu.add, Alu.mult)
                    nc.vector.tensor_tensor(
                        dsum[:, n * WT:(n + 1) * WT],
                        dsum[:, n * WT:(n + 1) * WT], d[:], Alu.add)

        # ---- V_ext: Vx = eT^T @ Wv via fp8 DoubleRow; Wv is pre-scaled
        # x32 on the host so its hi/lo fp8 split avoids e4m3 subnormals,
        # and the 1/32 is applied at readout.  3 cross terms; the lo*lo
        # term (~0.4%) is dropped. ----------------------------------------
        def pair2(t, pr, width, lo, hi):
            return t[:, pr * 2 * width:(pr + 1) * 2 * width].rearrange(
                "p (two w) -> p two w", two=2)[:, :, lo:hi]

        VTERMS = ((eT8h, Wv8h), (eT8h, Wv8l), (eT8l, Wv8h))
        for eblk in range(2):
            psv = [mono.tile([128, WH], f32, name="psv", tag="acc")
                   for _ in range(4)]
            for pr in range(NP):
                for ti, (lt, rt) in enumerate(VTERMS):
                    for e2 in range(2):
                        eo = eblk * 2 + e2
                        for n in range(NH):
                            nc.tensor.matmul(
                                psv[e2 * NH + n][:],
                                pair2(lt, pr, E, eo * 128, (eo + 1) * 128),
                                pair2(rt, pr, H, n * WH, (n + 1) * WH),
                                start=(pr == 0 and ti == 0),
                                stop=(pr == NP - 1 and ti == len(VTERMS) - 1),
                                perf_mode=DR)
            for e2 in range(2):
                eo = eblk * 2 + e2
                for n in range(NH):
                    ph = Vx8h[:, eo * H + n * WH: eo * H + (n + 1) * WH]
                    pl = Vx8l[:, eo * H + n * WH: eo * H + (n + 1) * WH]
                    nc.scalar.activation(ph, psv[e2 * NH + n][:], Act.Copy,
                                         bias=0.0, scale=1.0 / 32)
                    nc.vector.scalar_tensor_tensor(
                        pl, psv[e2 * NH + n][:], 1.0 / 32, ph,
                        Alu.mult, Alu.subtract)

        # s_self per token tile: column-sum dsum via tiny matmuls
        # (lhsT = dsum slice, rhs = ones column) -> [128 tokens, 1]
        for m in range(NT):
            ssp = mono.tile([128, 2], f32, name="ssp", tag="acc")
            nc.tensor.matmul(ssp[:], dsum[:, m * 128:(m + 1) * 128],
                             ones_c[:], start=True, stop=True)
            nc.vector.tensor_copy(ss_col[:, m:m + 1], ssp[:, 0:1])

        # tile 0's s_ext runs from the mono ring: it overlaps the V_ext
        # readout drain and the attention pool-open barrier
        ps_s0 = mono.tile([128, E], f32, name="ps_s0", tag="acc")
        for kc in range(KC):
            nc.tensor.matmul(
                ps_s0[:], QT[:, kc * T: kc * T + 128],
                KxT[:, kc * E:(kc + 1) * E],
                start=(kc == 0), stop=(kc == KC - 1))

      # ---- attention per token tile ------------------------------------
      # v_tok runs UNSCALED (xTb @ Wv, no softmax dependency) in its own
      # PSUM groups, filling the PE while the softmax chain computes; the
      # p_self scaling is applied per-partition at readout.  This removes
      # the p_self broadcast (transpose+matmul) chain entirely.
      with tc.tile_pool(name="ps_att", bufs=2, space="PSUM") as ps_att, \
           tc.tile_pool(name="ps_tr", bufs=2, space="PSUM") as ps_tr, \
           tc.tile_pool(name="ps_cu", bufs=1, space="PSUM") as ps_cu, \
           tc.tile_pool(name="work_a", bufs=2) as work_a, \
           tc.tile_pool(name="pt8", bufs=2) as pt8_pool:
        for m in range(NT):
            last = m == NT - 1
            # s_ext = Q^T.T @ K_ext^T  -> [128 tokens, E]  (f32r);
            # m=0 was computed from the mono ring above
            if m == 0:
                ps_s = ps_s0
            else:
                ps_s = ps_att.tile([128, E], f32, name="ps_s")
                for kc in range(KC):
                    nc.tensor.matmul(
                        ps_s[:],
                        QT[:, kc * T + m * 128: kc * T + (m + 1) * 128],
                        KxT[:, kc * E:(kc + 1) * E],
                        start=(kc == 0), stop=(kc == KC - 1))

            nmx = small.tile([128, 1], f32, name="nmx")
            nc.vector.tensor_reduce(nmx[:], ps_s[:], axis=X, op=Alu.max,
                                    negate=True)
            # nmx2 = min(-ss, nmx) = -max(s_self, max(s_ext))
            nmx2 = small.tile([128, 1], f32, name="nmx2")
            nc.vector.scalar_tensor_tensor(
                nmx2[:], ss_col[:, m:m + 1], -1.0, nmx[:],
                Alu.mult, Alu.min)
            # probs are computed x256 (bias += ln 256) so their fp8 hi/lo
            # split stays clear of e4m3 subnormals; 1/256 folds into r.
            nmx2b = small.tile([128, 1], f32, name="nmx2b")
            nc.vector.tensor_scalar_add(nmx2b[:], nmx2[:], 4.85203026)

            pe = work_a.tile([128, E], bf16, name="pe")
            Ze = small.tile([128, 1], f32, name="Ze")
            nc.scalar.activation(pe[:], ps_s[:], Act.Exp, bias=nmx2b[:],
                                 scale=1.0, accum_out=Ze[:])
            p_self = small.tile([128, 1], f32, name="p_self")
            nc.scalar.activation(p_self[:], ss_col[:, m:m + 1],
                                 Act.Exp, bias=nmx2[:], scale=1.0)
            Zt = small.tile([128, 1], f32, name="Zt")
            nc.vector.scalar_tensor_tensor(Zt[:], Ze[:], 1.0 / 128,
                                           p_self[:], Alu.mult, Alu.add)
            r = small.tile([128, 1], f32, name="r")
            nc.vector.reciprocal(r[:], Zt[:])
            pr = small.tile([128, 1], f32, name="pr")
            nc.vector.scalar_tensor_tensor(pr[:], p_self[:], 1.0 / 32,
                                           r[:], Alu.mult, Alu.mult)
            r256 = small.tile([128, 1], f32, name="r256")
            nc.vector.tensor_scalar_mul(r256[:], r[:], 1.0 / 128)

            # unscaled v_tok: psu[n] = (xTb slice).T @ Wv — independent of
            # the softmax, keeps the PE busy during the chain above
            psu = [ps_cu.tile([128, WH], f32, name=f"psu{n}", tag=f"u{n}")
                   for n in range(NH)]
            UTERMS = ((xT8h, Wv8h), (xT8h, Wv8l), (xT8l, Wv8h))
            for pr8 in range(NP):
                for ti, (lt, rt) in enumerate(UTERMS):
                    lhsT = pair2(lt, pr8, T, m * 128, (m + 1) * 128)
                    for n in range(NH):
                        nc.tensor.matmul(
                            psu[n][:], lhsT,
                            pair2(rt, pr8, H, n * WH, (n + 1) * WH),
                            start=(pr8 == 0 and ti == 0),
                            stop=(pr8 == NP - 1 and ti == len(UTERMS) - 1),
                            perf_mode=DR)

            # transpose unnormalized ext probs -> Pt (E-major, bf16),
            # then split hi/lo fp8 (copy on DVE, residual on Pool)
            Pt = work_a.tile([128, NE * 128], bf16, name="Pt")
            for ec in range(NE):
                pst = ps_tr.tile([128, 128], bf16, name="pst")
                nc.tensor.transpose(pst[:],
                                    pe[:, ec * 128:(ec + 1) * 128],
                                    identb[:])
                nc.vector.tensor_copy(Pt[:, ec * 128:(ec + 1) * 128],
                                      pst[:])
            Pt8h = pt8_pool.tile([128, NE * 128], f8, name="Pt8h")
            Pt8l = pt8_pool.tile([128, NE * 128], f8, name="Pt8l")
            nc.vector.tensor_copy(Pt8h[:], Pt[:])
            nc.gpsimd.tensor_tensor(Pt8l[:], Pt[:], Pt8h[:], Alu.subtract)

            # ctx_ext = Pt.T @ Vx
            psc = [ps_cu.tile([128, WH], f32, name=f"psc{n}", tag=f"c{n}")
                   for n in range(NH)]
            CTERMS = ((Pt8h, Vx8h), (Pt8h, Vx8l), (Pt8l, Vx8h))
            NEP = NE // 2  # ec-pairs

            def ctx_mm(n, ep, ti):
                lt, rt = CTERMS[ti]
                nc.tensor.matmul(
                    psc[n][:],
                    lt[:].rearrange("p (ep two e) -> p ep two e",
                                    ep=NEP, two=2)[:, ep],
                    rt[:, 2 * ep * H:(2 * ep + 2) * H].rearrange(
                        "p (two h) -> p two h",
                        two=2)[:, :, n * WH:(n + 1) * WH],
                    start=(ep == 0 and ti == 0),
                    stop=(ep == NEP - 1 and ti == len(CTERMS) - 1),
                    perf_mode=DR)

            if not last:
                for ep in range(NEP):
                    for ti in range(len(CTERMS)):
                        for n in range(NH):
                            ctx_mm(n, ep, ti)
            else:
                # finish chunk 1 first so its readout+store overlaps
                # chunk 0's matmuls and the drain is one chunk shorter
                for n in (1, 0):
                    for ep in range(NEP):
                        for ti in range(len(CTERMS)):
                            ctx_mm(n, ep, ti)

            # out = r*ctx_ext + (p_self*r)*v_tok + bvb, stored in strips
            order = (1, 0) if last else tuple(range(NH))
            osbs = {}
            # psu stops well before ctx_ext: emit both psu folds first so
            # the post-matmul DVE chain is only the ctx folds
            for n in order:
                osbs[n] = work_a.tile([128, WH], f32, name="osb")
                nc.vector.scalar_tensor_tensor(
                    osbs[n][:], psu[n][:],
                    pr[:], bvb[:, n * WH:(n + 1) * WH],
                    Alu.mult, Alu.add)
            for n in order:
                nc.vector.scalar_tensor_tensor(
                    osbs[n][:], psc[n][:],
                    r256[:], osbs[n][:],
                    Alu.mult, Alu.add)
                dma_q = nc.sync if (last and n == 0) else nc.scalar
                dma_q.dma_start(
                    out_d[m * 128:(m + 1) * 128, n * WH:(n + 1) * WH],
                    osbs[n][:])


def _build_module(T, H, E, reps=1):
    from contextlib import ExitStack
    import concourse.tile as tile
    from concourse import bacc

    nc = bacc.Bacc(None)
    with ExitStack() as ctx:
        tc = ctx.enter_context(tile.TileContext(nc))
        _emit(nc, tc, ctx, T, H, E, reps)
    nc.finalize()
    return nc


# --------------------------------------------------------------------------
# host side
# --------------------------------------------------------------------------

def _shard_inputs(hidden_states, external_embeddings, Wq, bq, Wk, bk, Wv, bv):
    """Build the per-core input maps (host-side layout prep)."""
    hs = np.asarray(hidden_states, dtype=np.float32)
    ext = np.asarray(external_embeddings, dtype=np.float32)
    Wq = np.ascontiguousarray(np.asarray(Wq, dtype=np.float32))
    Wk = np.ascontiguousarray(np.asarray(Wk, dtype=np.float32))
    Wv = np.asarray(Wv, dtype=np.float32)
    bq = np.asarray(bq, dtype=np.float32)
    bk = np.asarray(bk, dtype=np.float32)
    bv = np.asarray(bv, dtype=np.float32)

    f8 = ml_dtypes.float8_e4m3

    def fp8_pairs(a):
        """[H, N] f32 -> hi/lo fp8 arrays [H//2, 2*N] in DoubleRow
        kc-pair layout: row pr*128+p holds chunks (2pr, 2pr+1)."""
        Hd, N = a.shape
        hi = a.astype(f8)
        lo = (a - hi.astype(np.float32)).astype(f8)
        out = []
        for arr in (hi, lo):
            v = arr.reshape(Hd // 256, 2, 128, N).transpose(0, 2, 1, 3)
            out.append(np.ascontiguousarray(v.reshape(Hd // 2, 2 * N)))
        return out

    Wv8h, Wv8l = fp8_pairs(Wv * 32.0)

    KC = H // 128
    bqc = np.ascontiguousarray(bq.reshape(KC, 128).T)  # [128, KC]
    bkc = np.ascontiguousarray(bk.reshape(KC, 128).T)
    bvr = np.ascontiguousarray(bv.reshape(1, H).astype(ml_dtypes.bfloat16))

    flat = hs.reshape(B * S, H)
    in_maps = []
    _ET8 = {}
    for c in range(NCORES):
        b = (c * T) // S
        xT = np.ascontiguousarray(flat[c * T:(c + 1) * T, :].T)  # [H, T]
        eT = np.ascontiguousarray(ext[b].T)                      # [H, E]
        eT8h, eT8l = _ET8.setdefault(b, fp8_pairs(eT))
        xT8h, xT8l = fp8_pairs(xT)
        in_maps.append({
            "xT": xT, "xT8h": xT8h, "xT8l": xT8l,
            "eT": eT, "eT8h": eT8h, "eT8l": eT8l,
            "Wq": Wq, "Wk": Wk, "Wv8h": Wv8h, "Wv8l": Wv8l,
            "bqc": bqc, "bkc": bkc, "bvr": bvr,
            "onesl": _ONESL, "onesc": _ONESC, "eye": _EYEB,
        })
    return in_maps


def kernel(hidden_states, external_embeddings, Wq, bq, Wk, bk, Wv, bv):
    from concourse.bass_utils import run_bass_kernel_spmd

    key = "main"
    if key not in _RUNNER_CACHE:
        _RUNNER_CACHE[key] = _build_module(T, H, E)
    nc = _RUNNER_CACHE[key]

    in_maps = _shard_inputs(hidden_states, external_embeddings,
                            Wq, bq, Wk, bk, Wv, bv)
    res = run_bass_kernel_spmd(nc, in_maps, list(range(NCORES)))
    out = np.concatenate([res.results[c]["out"] for c in range(NCORES)],
                         axis=0)
    return out.reshape(B, S, H)


# revision 58
# speedup vs baseline: 3.3966x; 1.0068x over previous
"""Trainium2 Bass kernel for ExternalEmbeddingSelfAttention.

Computation (per batch b):
    q     = hs @ Wq + bq           [S,H]
    k_tok = hs @ Wk + bk           [S,H]
    v_tok = hs @ Wv + bv           [S,H]
    k_ext = ext @ Wk + bk          [E,H]
    v_ext = ext @ Wv + bv          [E,H]
    s_self[t] = q[t] . k_tok[t]                (per-token self score)
    s_ext = q @ k_ext^T            [S,E]
    probs = softmax([s_self, s_ext], axis=-1)  (no 1/sqrt(d) scaling)
    out   = probs[:,0:1]*v_tok + probs[:,1:] @ v_ext

Sharding: 8 cores, each takes 1024 contiguous tokens of the flattened
(B*S, H) token axis (core i -> batch i//2, S-half i%2).  Each core also
computes its batch's external projections (duplicated across the 2 cores
sharing a batch).

Device schedule (per core, T=1024):  the DMA device is a single shared
332 GB/s resource, so every input DMA is issued on ONE queue (sync) in
exactly first-needed-first order; weights stream in natural row-chunk
layout (contiguous).  Phases:
  QA  (ho 0..3, kc-outer, full T): needs xT[kc]+WqA[kc] just-in-time;
      first matmul starts ~6us after launch instead of ~19us.
  QB  (ho 4..7): xT resident, WqB[kc] streams; Wk/eT prefetch behind.
  K_ext (kc-outer): Wk[kc]+eT[kc] mostly prefetched; KxT = Wk^T@eT+bk.
  K_tok+s_self (ho-outer): Wk/xT resident; d=(k+bk)*q accumulated on
      DVE into dsum, single ones-matmul -> s_self (saves PE work).
  V_ext (kc-outer): Vx = eT^T @ Wv (+0), stored bf16.
  Attention per 128-token tile: s_ext = QT.T@KxT (f32r); softmax with
      self column folded in; unnormalized probs PE-transposed; ctx PSUM
      accumulates ext context (Pt.T@Vx) and the self term
      ((xT*p_self_bcast).T@Wv); readout applies 1/Z and bv, split per
      512-col chunk so the last store overlaps the last readout.

Wv/Vx are bf16 (same PE rate, half DMA/SBUF; value-path precision is
ample).  Score pipeline stays f32r (FP22) end to end.
"""

import numpy as np
import ml_dtypes

B, S, E, H = 4, 2048, 512, 1024
NCORES = 8
T = (B * S) // NCORES  # tokens per core = 1024

_RUNNER_CACHE = {}

_ONESL = np.ones((1, 128), dtype=ml_dtypes.bfloat16)
_ONESC = np.ones((128, 2), dtype=np.float32)
_EYEB = np.eye(128, dtype=ml_dtypes.bfloat16)


# --------------------------------------------------------------------------
# device kernel emission
# --------------------------------------------------------------------------

def _emit(nc, tc, ctx, T, H, E, reps=1):
    import contextlib
    import concourse.mybir as mybir

    f32 = mybir.dt.float32
    f32r = mybir.dt.float32r
    bf16 = mybir.dt.bfloat16
    f8 = mybir.dt.float8e4
    DR = mybir.MatmulPerfMode.DoubleRow
    Alu = mybir.AluOpType
    Act = mybir.ActivationFunctionType
    X = mybir.AxisListType.X

    KC = H // 128          # contraction chunks over h_in; also h_out tiles
    NT = T // 128          # token tiles
    NE = E // 128          # ext tiles
    WH = min(512, H)       # h_out free-dim chunk
    NH = H // WH
    WT = min(512, T)       # token free-dim chunk
    NTW = T // WT
    assert E <= 512 and NTW == 2 and NH == 2 and KC == 8

    xT_d = nc.declare_dram_parameter("xT", [H, T], f32, isOutput=False)
    eT_d = nc.declare_dram_parameter("eT", [H, E], f32, isOutput=False)
    eT8h_d = nc.declare_dram_parameter("eT8h", [H // 2, 2 * E], f8,
                                       isOutput=False)
    eT8l_d = nc.declare_dram_parameter("eT8l", [H // 2, 2 * E], f8,
                                       isOutput=False)
    xT8h_d = nc.declare_dram_parameter("xT8h", [H // 2, 2 * T], f8,
                                       isOutput=False)
    xT8l_d = nc.declare_dram_parameter("xT8l", [H // 2, 2 * T], f8,
                                       isOutput=False)
    wq_d = nc.declare_dram_parameter("Wq", [H, H], f32, isOutput=False)
    wk_d = nc.declare_dram_parameter("Wk", [H, H], f32, isOutput=False)
    wv8h_d = nc.declare_dram_parameter("Wv8h", [H // 2, 2 * H], f8,
                                       isOutput=False)
    wv8l_d = nc.declare_dram_parameter("Wv8l", [H // 2, 2 * H], f8,
                                       isOutput=False)
    bqc_d = nc.declare_dram_parameter("bqc", [128, KC], f32, isOutput=False)
    bkc_d = nc.declare_dram_parameter("bkc", [128, KC], f32, isOutput=False)
    bvr_d = nc.declare_dram_parameter("bvr", [1, H], bf16, isOutput=False)
    onesl_d = nc.declare_dram_parameter("onesl", [1, 128], bf16, isOutput=False)
    onesc_d = nc.declare_dram_parameter("onesc", [128, 2], f32, isOutput=False)
    eye_d = nc.declare_dram_parameter("eye", [128, 128], bf16, isOutput=False)
    out_d = nc.declare_dram_parameter("out", [T, H], f32, isOutput=True)

    cp = ctx.enter_context(tc.tile_pool(name="cp", bufs=1))
    small = ctx.enter_context(tc.tile_pool(name="small", bufs=3))

    # ---- persistent SBUF tensors -----------------------------------------
    QT = cp.tile([128, KC * T], f32r, name="QT")    # Q^T hidden-major
    KxT = cp.tile([128, KC * E], f32r, name="KxT")  # K_ext^T hidden-major
    Vx8h = cp.tile([128, NE * H], f8, name="Vx8h")  # V_ext E-major fp8 hi
    Vx8l = cp.tile([128, NE * H], f8, name="Vx8l")  # and lo residual
    xT = cp.tile([128, KC * T], f32r, name="xT")    # chunk kc: cols [kc*T,+T)
    Wk = cp.tile([128, KC * H], f32r, name="Wk")    # Wk rows, chunk-major
    eT = cp.tile([128, KC * E], f32r, name="eT")
    NP = KC // 2  # DoubleRow kc-pairs
    Wv8h = cp.tile([128, NP * 2 * H], f8, name="Wv8h")  # 32*Wv hi/lo fp8
    Wv8l = cp.tile([128, NP * 2 * H], f8, name="Wv8l")
    eT8h = cp.tile([128, NP * 2 * E], f8, name="eT8h")
    eT8l = cp.tile([128, NP * 2 * E], f8, name="eT8l")
    xT8h = cp.tile([128, NP * 2 * T], f8, name="xT8h")
    xT8l = cp.tile([128, NP * 2 * T], f8, name="xT8l")
    dsum = cp.tile([128, T], f32r, name="dsum")     # per-chunk q*k partials
    ss_col = cp.tile([128, NT], f32, name="ss_col")
    bqc = cp.tile([128, KC], f32, name="bqc")
    bkc = cp.tile([128, KC], f32, name="bkc")
    bvr = cp.tile([1, H], bf16, name="bvr")
    bvb = cp.tile([128, H], bf16, name="bvb")
    ones_c = cp.tile([128, 2], f32r, name="ones_c")
    identb = cp.tile([128, 128], bf16, name="identb")

    loop_cm = tc.For_i(0, reps, 1) if reps > 1 else contextlib.nullcontext()
    with loop_cm:
      with tc.tile_pool(name="mono", bufs=8, space="PSUM") as mono, \
           tc.tile_pool(name="work_k", bufs=1) as work_k:
        with tc.tile_pool(name="wq", bufs=4) as wq_pool:
            wqa = [wq_pool.tile([128, WT], f32r, name="wqa") for _ in range(KC)]
            wqb = [wq_pool.tile([128, WT], f32r, name="wqb") for _ in range(KC)]

            # ---- the ordered input stream (single queue = device order) --
            for kc in range(KC):
                nc.sync.dma_start(xT[:, kc * T:(kc + 1) * T],
                                  xT_d[kc * 128:(kc + 1) * 128, :].bitcast(f32r))
                nc.sync.dma_start(
                    wqa[kc][:],
                    wq_d[kc * 128:(kc + 1) * 128, 0:WT].bitcast(f32r))
                if kc == 2:
                    nc.sync.dma_start(bqc[:], bqc_d[:])
                    nc.sync.dma_start(bkc[:], bkc_d[:])
            for kc in range(KC):
                nc.sync.dma_start(
                    wqb[kc][:],
                    wq_d[kc * 128:(kc + 1) * 128, WT:H].bitcast(f32r))
            for kc in range(KC):
                nc.sync.dma_start(Wk[:, kc * H:(kc + 1) * H],
                                  wk_d[kc * 128:(kc + 1) * 128, :].bitcast(f32r))
                nc.sync.dma_start(eT[:, kc * E:(kc + 1) * E],
                                  eT_d[kc * 128:(kc + 1) * 128, :].bitcast(f32r))
            nc.sync.dma_start(ones_c[:], onesc_d[:].bitcast(f32r))
            nc.sync.dma_start(bvr[:], bvr_d[:])
            nc.sync.dma_start(ones_lb[:], onesl_d[:])
            nc.sync.dma_start(identb[:], eye_d[:])
            for pr in range(NP):
                nc.sync.dma_start(Wv8h[:, pr * 2 * H:(pr + 1) * 2 * H],
                                  wv8h_d[pr * 128:(pr + 1) * 128, :])
                nc.sync.dma_start(Wv8l[:, pr * 2 * H:(pr + 1) * 2 * H],
                                  wv8l_d[pr * 128:(pr + 1) * 128, :])
            for pr in range(NP):
                nc.sync.dma_start(eT8h[:, pr * 2 * E:(pr + 1) * 2 * E],
                                  eT8h_d[pr * 128:(pr + 1) * 128, :])
                nc.sync.dma_start(eT8l[:, pr * 2 * E:(pr + 1) * 2 * E],
                                  eT8l_d[pr * 128:(pr + 1) * 128, :])
            for pr in range(NP):
                nc.sync.dma_start(xT8h[:, pr * 2 * T:(pr + 1) * 2 * T],
                                  xT8h_d[pr * 128:(pr + 1) * 128, :])
                nc.sync.dma_start(xT8l[:, pr * 2 * T:(pr + 1) * 2 * T],
                                  xT8l_d[pr * 128:(pr + 1) * 128, :])


            # preload the Exp activation table off the critical path
            dummy = small.tile([1, 2], f32, name="dummy")
            nc.scalar.memzero(dummy[:])
            nc.scalar.activation(dummy[:], dummy[:], Act.Exp, bias=0.0,
                                 scale=1.0)

            # ---- QA / QB: QT = Wq^T @ xT (+bq), kc-outer, half-ho --------
            # All pre-attention phases share the mono PSUM pool: bank
            # handoff is a per-bank WAR dep (phase N+1's start=True matmul
            # on a bank waits only that bank's readout), not a pool barrier.
            for half, wqs in ((0, wqa), (1, wqb)):
                psq = [mono.tile([128, WT], f32, name="psq", tag="acc")
                       for _ in range(8)]
                for kc in range(KC):
                    for ho4 in range(4):
                        for n in range(NTW):
                            nc.tensor.matmul(
                                psq[ho4 * NTW + n][:],
                                wqs[kc][:, ho4 * 128:(ho4 + 1) * 128],
                                xT[:, kc * T + n * WT: kc * T + (n + 1) * WT],
                                start=(kc == 0), stop=(kc == KC - 1))
                for ho4 in range(4):
                    ho = half * 4 + ho4
                    for n in range(NTW):
                        nc.vector.tensor_scalar_add(
                            QT[:, ho * T + n * WT: ho * T + (n + 1) * WT],
                            psq[ho4 * NTW + n][:], bqc[:, ho:ho + 1])

        # ---- K_ext: KxT = Wk^T @ eT (+bk), kc-outer, 2 sub-phases --------
        for hblk in range(2):
            pske = [mono.tile([128, E], f32, name="pske", tag="acc") for _ in range(4)]
            for kc in range(KC):
                for h4 in range(4):
                    ho = hblk * 4 + h4
                    nc.tensor.matmul(
                        pske[h4][:],
                        Wk[:, kc * H + ho * 128: kc * H + (ho + 1) * 128],
                        eT[:, kc * E:(kc + 1) * E],
                        start=(kc == 0), stop=(kc == KC - 1))
            for h4 in range(4):
                ho = hblk * 4 + h4
                nc.vector.tensor_scalar_add(KxT[:, ho * E:(ho + 1) * E],
                                            pske[h4][:], bkc[:, ho:ho + 1])

        # ---- K_tok + s_self (ho-outer; Wk/xT resident) -------------------
        # bvb = bv broadcast to all partitions (built here, PSUM has slack)
        for n in range(NH):
            pbv = mono.tile([128, WH], f32, name="pbv", tag="acc")
            nc.tensor.matmul(pbv[:], ones_lb[:],
                             bvr[:, n * WH:(n + 1) * WH],
                             start=True, stop=True)
            nc.vector.tensor_copy(bvb[:, n * WH:(n + 1) * WH], pbv[:])
        for ho in range(KC):
            pskt = [mono.tile([128, WT], f32, name="pskt", tag="acc")
                    for _ in range(NTW)]
            for kc in range(KC):
                for n in range(NTW):
                    nc.tensor.matmul(
                        pskt[n][:],
                        Wk[:, kc * H + ho * 128: kc * H + (ho + 1) * 128],
                        xT[:, kc * T + n * WT: kc * T + (n + 1) * WT],
                        start=(kc == 0), stop=(kc == KC - 1))
            # d = (k_tok^T + bk) * QT, straight from PSUM; accumulate
            # into dsum on DVE (cheaper than per-ho PE ones-matmuls)
            if ho == 0:
                for n in range(NTW):
                    nc.vector.scalar_tensor_tensor(
                        dsum[:, n * WT:(n + 1) * WT], pskt[n][:],
                        bkc[:, ho:ho + 1],
                        QT[:, ho * T + n * WT: ho * T + (n + 1) * WT],
                        Alu.add, Alu.mult)
            else:
                for n in range(NTW):
                    d = work_k.tile([128, WT], f32r, name="d")
                    nc.vector.scalar_tensor_tensor(
                        d[:], pskt[n][:], bkc[:, ho:ho + 1],
                        QT[:, ho * T + n * WT: ho * T + (n + 1) * WT],
                        Alu.add, Alu.mult)
                    nc.vector.tensor_tensor(
                        dsum[:, n * WT:(n + 1) * WT],
                        dsum[:, n * WT:(n + 1) * WT], d[:], Alu.add)

        # ---- V_ext: Vx = eT^T @ Wv via fp8 DoubleRow; Wv is pre-scaled
        # x32 on the host so its hi/lo fp8 split avoids e4m3 subnormals,
        # and the 1/32 is applied at readout.  3 cross terms; the lo*lo
        # term (~0.4%) is dropped. ----------------------------------------
        def pair2(t, pr, width, lo, hi):
            return t[:, pr * 2 * width:(pr + 1) * 2 * width].rearrange(
                "p (two w) -> p two w", two=2)[:, :, lo:hi]

        VTERMS = ((eT8h, Wv8h), (eT8h, Wv8l), (eT8l, Wv8h))
        for eblk in range(2):
            psv = [mono.tile([128, WH], f32, name="psv", tag="acc")
                   for _ in range(4)]
            for pr in range(NP):
                for ti, (lt, rt) in enumerate(VTERMS):
                    for e2 in range(2):
                        eo = eblk * 2 + e2
                        for n in range(NH):
                            nc.tensor.matmul(
                                psv[e2 * NH + n][:],
                                pair2(lt, pr, E, eo * 128, (eo + 1) * 128),
                                pair2(rt, pr, H, n * WH, (n + 1) * WH),
                                start=(pr == 0 and ti == 0),
                                stop=(pr == NP - 1 and ti == len(VTERMS) - 1),
                                perf_mode=DR)
            for e2 in range(2):
                eo = eblk * 2 + e2
                for n in range(NH):
                    ph = Vx8h[:, eo * H + n * WH: eo * H + (n + 1) * WH]
                    pl = Vx8l[:, eo * H + n * WH: eo * H + (n + 1) * WH]
                    nc.scalar.activation(ph, psv[e2 * NH + n][:], Act.Copy,
                                         bias=0.0, scale=1.0 / 32)
                    nc.vector.scalar_tensor_tensor(
                        pl, psv[e2 * NH + n][:], 1.0 / 32, ph,
                        Alu.mult, Alu.subtract)

        # s_self per token tile: column-sum dsum via tiny matmuls
        # (lhsT = dsum slice, rhs = ones column) -> [128 tokens, 1]
        for m in range(NT):
            ssp = mono.tile([128, 2], f32, name="ssp", tag="acc")
            nc.tensor.matmul(ssp[:], dsum[:, m * 128:(m + 1) * 128],
                             ones_c[:], start=True, stop=True)
            nc.vector.tensor_copy(ss_col[:, m:m + 1], ssp[:, 0:1])

        # tile 0's s_ext runs from the mono ring: it overlaps the V_ext
        # readout drain and the attention pool-open barrier
        ps_s0 = mono.tile([128, E], f32, name="ps_s0", tag="acc")
        for kc in range(KC):
            nc.tensor.matmul(
                ps_s0[:], QT[:, kc * T: kc * T + 128],
                KxT[:, kc * E:(kc + 1) * E],
                start=(kc == 0), stop=(kc == KC - 1))

      # ---- attention per token tile ------------------------------------
      # v_tok runs UNSCALED (xTb @ Wv, no softmax dependency) in its own
      # PSUM groups, filling the PE while the softmax chain computes; the
      # p_self scaling is applied per-partition at readout.  This removes
      # the p_self broadcast (transpose+matmul) chain entirely.
      with tc.tile_pool(name="ps_att", bufs=2, space="PSUM") as ps_att, \
           tc.tile_pool(name="ps_tr", bufs=2, space="PSUM") as ps_tr, \
           tc.tile_pool(name="ps_cu", bufs=1, space="PSUM") as ps_cu, \
           tc.tile_pool(name="work_a", bufs=3) as work_a, \
           tc.tile_pool(name="pt8", bufs=2) as pt8_pool:
        for m in range(NT):
            last = m == NT - 1
            # s_ext = Q^T.T @ K_ext^T  -> [128 tokens, E]  (f32r);
            # m=0 was computed from the mono ring above
            if m == 0:
                ps_s = ps_s0
            else:
                ps_s = ps_att.tile([128, E], f32, name="ps_s")
                for kc in range(KC):
                    nc.tensor.matmul(
                        ps_s[:],
                        QT[:, kc * T + m * 128: kc * T + (m + 1) * 128],
                        KxT[:, kc * E:(kc + 1) * E],
                        start=(kc == 0), stop=(kc == KC - 1))

            nmx = small.tile([128, 1], f32, name="nmx")
            nc.vector.tensor_reduce(nmx[:], ps_s[:], axis=X, op=Alu.max,
                                    negate=True)
            # nmx2 = min(-ss, nmx) = -max(s_self, max(s_ext))
            nmx2 = small.tile([128, 1], f32, name="nmx2")
            nc.vector.scalar_tensor_tensor(
                nmx2[:], ss_col[:, m:m + 1], -1.0, nmx[:],
                Alu.mult, Alu.min)
            # probs are computed x256 (bias += ln 256) so their fp8 hi/lo
            # split stays clear of e4m3 subnormals; 1/256 folds into r.
            nmx2b = small.tile([128, 1], f32, name="nmx2b")
            nc.vector.tensor_scalar_add(nmx2b[:], nmx2[:], 4.85203026)

            pe = work_a.tile([128, E], bf16, name="pe")
            Ze = small.tile([128, 1], f32, name="Ze")
            nc.scalar.activation(pe[:], ps_s[:], Act.Exp, bias=nmx2b[:],
                                 scale=1.0, accum_out=Ze[:])
            p_self = small.tile([128, 1], f32, name="p_self")
            nc.scalar.activation(p_self[:], ss_col[:, m:m + 1],
                                 Act.Exp, bias=nmx2[:], scale=1.0)
            Zt = small.tile([128, 1], f32, name="Zt")
            nc.vector.scalar_tensor_tensor(Zt[:], Ze[:], 1.0 / 128,
                                           p_self[:], Alu.mult, Alu.add)
            r = small.tile([128, 1], f32, name="r")
            nc.vector.reciprocal(r[:], Zt[:])
            pr = small.tile([128, 1], f32, name="pr")
            nc.vector.scalar_tensor_tensor(pr[:], p_self[:], 1.0 / 32,
                                           r[:], Alu.mult, Alu.mult)
            r256 = small.tile([128, 1], f32, name="r256")
            nc.vector.tensor_scalar_mul(r256[:], r[:], 1.0 / 128)

            # unscaled v_tok: psu[n] = (xTb slice).T @ Wv — independent of
            # the softmax, keeps the PE busy during the chain above
            psu = [ps_cu.tile([128, WH], f32, name=f"psu{n}", tag=f"u{n}")
                   for n in range(NH)]
            UTERMS = ((xT8h, Wv8h), (xT8h, Wv8l), (xT8l, Wv8h))
            for pr8 in range(NP):
                for ti, (lt, rt) in enumerate(UTERMS):
                    lhsT = pair2(lt, pr8, T, m * 128, (m + 1) * 128)
                    for n in range(NH):
                        nc.tensor.matmul(
                            psu[n][:], lhsT,
                            pair2(rt, pr8, H, n * WH, (n + 1) * WH),
                            start=(pr8 == 0 and ti == 0),
                            stop=(pr8 == NP - 1 and ti == len(UTERMS) - 1),
                            perf_mode=DR)

            # transpose unnormalized ext probs -> Pt (E-major, bf16),
            # then split hi/lo fp8 (copy on DVE, residual on Pool)
            Pt = work_a.tile([128, NE * 128], bf16, name="Pt")
            for ec in range(NE):
                pst = ps_tr.tile([128, 128], bf16, name="pst")
                nc.tensor.transpose(pst[:],
                                    pe[:, ec * 128:(ec + 1) * 128],
                                    identb[:])
                nc.vector.tensor_copy(Pt[:, ec * 128:(ec + 1) * 128],
                                      pst[:])
            Pt8h = pt8_pool.tile([128, NE * 128], f8, name="Pt8h")
            Pt8l = pt8_pool.tile([128, NE * 128], f8, name="Pt8l")
            nc.vector.tensor_copy(Pt8h[:], Pt[:])
            nc.gpsimd.tensor_tensor(Pt8l[:], Pt[:], Pt8h[:], Alu.subtract)

            # ctx_ext = Pt.T @ Vx
            psc = [ps_cu.tile([128, WH], f32, name=f"psc{n}", tag=f"c{n}")
                   for n in range(NH)]
            CTERMS = ((Pt8h, Vx8h), (Pt8h, Vx8l), (Pt8l, Vx8h))
            NEP = NE // 2  # ec-pairs

            def ctx_mm(n, ep, ti):
                lt, rt = CTERMS[ti]
                nc.tensor.matmul(
                    psc[n][:],
                    lt[:].rearrange("p (ep two e) -> p ep two e",
                                    ep=NEP, two=2)[:, ep],
                    rt[:, 2 * ep * H:(2 * ep + 2) * H].rearrange(
                        "p (two h) -> p two h",
                        two=2)[:, :, n * WH:(n + 1) * WH],
                    start=(ep == 0 and ti == 0),
                    stop=(ep == NEP - 1 and ti == len(CTERMS) - 1),
                    perf_mode=DR)

            if not last:
                for ep in range(NEP):
                    for ti in range(len(CTERMS)):
                        for n in range(NH):
                            ctx_mm(n, ep, ti)
            else:
                # finish chunk 1 first so its readout+store overlaps
                # chunk 0's matmuls and the drain is one chunk shorter
                for n in (1, 0):
                    for ep in range(NEP):
                        for ti in range(len(CTERMS)):
                            ctx_mm(n, ep, ti)

            # out = r*ctx_ext + (p_self*r)*v_tok + bvb, stored in strips
            order = (1, 0) if last else tuple(range(NH))
            osbs = {}
            # psu stops well before ctx_ext: emit both psu folds first so
            # the post-matmul DVE chain is only the ctx folds
            for n in order:
                osbs[n] = work_a.tile([128, WH], f32, name="osb")
                nc.vector.scalar_tensor_tensor(
                    osbs[n][:], psu[n][:],
                    pr[:], bvb[:, n * WH:(n + 1) * WH],
                    Alu.mult, Alu.add)
            for n in order:
                nc.vector.scalar_tensor_tensor(
                    osbs[n][:], psc[n][:],
                    r256[:], osbs[n][:],
                    Alu.mult, Alu.add)
                dma_q = nc.sync if (last and n == 0) else nc.scalar
                dma_q.dma_start(
                    out_d[m * 128:(m + 1) * 128, n * WH:(n + 1) * WH],
                    osbs[n][:])


def _build_module(T, H, E, reps=1):
    from contextlib import ExitStack
    import concourse.tile as tile
    from concourse import bacc

    nc = bacc.Bacc(None)
    with ExitStack() as ctx:
        tc = ctx.enter_context(tile.TileContext(nc))
        _emit(nc, tc, ctx, T, H, E, reps)
    nc.finalize()
    return nc


# --------------------------------------------------------------------------
# host side
# --------------------------------------------------------------------------

def _shard_inputs(hidden_states, external_embeddings, Wq, bq, Wk, bk, Wv, bv):
    """Build the per-core input maps (host-side layout prep)."""
    hs = np.asarray(hidden_states, dtype=np.float32)
    ext = np.asarray(external_embeddings, dtype=np.float32)
    Wq = np.ascontiguousarray(np.asarray(Wq, dtype=np.float32))
    Wk = np.ascontiguousarray(np.asarray(Wk, dtype=np.float32))
    Wv = np.asarray(Wv, dtype=np.float32)
    bq = np.asarray(bq, dtype=np.float32)
    bk = np.asarray(bk, dtype=np.float32)
    bv = np.asarray(bv, dtype=np.float32)

    f8 = ml_dtypes.float8_e4m3

    def fp8_pairs(a):
        """[H, N] f32 -> hi/lo fp8 arrays [H//2, 2*N] in DoubleRow
        kc-pair layout: row pr*128+p holds chunks (2pr, 2pr+1)."""
        Hd, N = a.shape
        hi = a.astype(f8)
        lo = (a - hi.astype(np.float32)).astype(f8)
        out = []
        for arr in (hi, lo):
            v = arr.reshape(Hd // 256, 2, 128, N).transpose(0, 2, 1, 3)
            out.append(np.ascontiguousarray(v.reshape(Hd // 2, 2 * N)))
        return out

    Wv8h, Wv8l = fp8_pairs(Wv * 32.0)

    KC = H // 128
    bqc = np.ascontiguousarray(bq.reshape(KC, 128).T)  # [128, KC]
    bkc = np.ascontiguousarray(bk.reshape(KC, 128).T)
    bvr = np.ascontiguousarray(bv.reshape(1, H).astype(ml_dtypes.bfloat16))

    flat = hs.reshape(B * S, H)
    in_maps = []
    _ET8 = {}
    for c in range(NCORES):
        b = (c * T) // S
        xT = np.ascontiguousarray(flat[c * T:(c + 1) * T, :].T)  # [H, T]
        eT = np.ascontiguousarray(ext[b].T)                      # [H, E]
        eT8h, eT8l = _ET8.setdefault(b, fp8_pairs(eT))
        xT8h, xT8l = fp8_pairs(xT)
        in_maps.append({
            "xT": xT, "xT8h": xT8h, "xT8l": xT8l,
            "eT": eT, "eT8h": eT8h, "eT8l": eT8l,
            "Wq": Wq, "Wk": Wk, "Wv8h": Wv8h, "Wv8l": Wv8l,
            "bqc": bqc, "bkc": bkc, "bvr": bvr,
            "onesl": _ONESL, "onesc": _ONESC, "eye": _EYEB,
        })
    return in_maps


def kernel(hidden_states, external_embeddings, Wq, bq, Wk, bk, Wv, bv):
    from concourse.bass_utils import run_bass_kernel_spmd

    key = "main"
    if key not in _RUNNER_CACHE:
        _RUNNER_CACHE[key] = _build_module(T, H, E)
    nc = _RUNNER_CACHE[key]

    in_maps = _shard_inputs(hidden_states, external_embeddings,
                            Wq, bq, Wk, bk, Wv, bv)
    res = run_bass_kernel_spmd(nc, in_maps, list(range(NCORES)))
    out = np.concatenate([res.results[c]["out"] for c in range(NCORES)],
                         axis=0)
    return out.reshape(B, S, H)
